# revision 12
# baseline (speedup 1.0000x reference)
"""BatchedGCN Trainium2 kernel.

Per graph (batch element):
  norms_i = ||X_i||;  A = (X@X.T > 0.3*n_i*n_j) + I ; deg = rowsum(A); d = deg^-1/2
  H1 = relu(diag(d) A diag(d) (X @ W1.T) + b1)
  H2 = diag(d) A diag(d) (H1 @ W2.T) + b2
  out = H2 / max(||H2_row||, 1e-12)

Key implementation choices:
- The cosine threshold runs in un-normalized form:
  Xn_i . Xn_j > t  <=>  (X_i . X_j) * (1/max(n_i,eps)) / t > n_j.
  The diag(norm) factor relating X to Xn cancels against the un-normalized
  X used in the first linear layer, so the output path needs no norms.
- The gram matrix G = X X^T runs in fp8 (DoubleRow, 2x rate); the
  thresholding margin is ~40% of the bound while fp8 dot-product error is
  <0.5%, so A is bit-exact.  Row norms are read off G's diagonal blocks
  (computed in a cheap per-row-tile pre-pass), so they are fp8-accurate -
  again only used for the threshold bound.
- The two propagations and both linear layers run in bf16 with fp32 PSUM.
- Sharding: data-parallel over B=32 across 8 cores (4 graphs each),
  weights replicated.  Host-side layout prep ships X^T pre-cast (bf16 and
  DoubleRow-packed fp8) and transposed weights, so the kernel needs no
  on-chip transposes or casts.
- Engine balance: PSUM evictions are spread over DVE / Pool / Act so the
  tensor engine is the only near-saturated engine; all DMA descriptor
  generation is on the SP (sync) hardware DGE, none on Pool.
- Phase waves: every phase is emitted for all resident graphs before the
  next phase, so each graph's latency chains (norm -> DRAM-bounce
  broadcast -> threshold, psum eviction chains) hide behind the other
  graphs' dense matmuls and the PE stays continuously busy (p-state).
"""

from contextlib import ExitStack

import ml_dtypes
import numpy as np

import concourse.bass as bass
import concourse.mybir as mybir
import concourse.tile as tile
from concourse import bacc
from concourse.bass_utils import run_bass_kernel_spmd
from concourse.masks import make_identity

B, N, D_IN, D_H, D_OUT = 32, 1024, 768, 256, 128
N_CORES = 8
BPC = B // N_CORES          # graphs per core
NT = N // 128               # 8 row tiles
DTI = D_IN // 128           # 6 input-dim tiles
HC = D_H // 128             # 2 hidden chunks
KDR = D_IN // 256           # 3 DoubleRow K-chunks
F32 = mybir.dt.float32
BF16 = mybir.dt.bfloat16
FP8 = mybir.dt.float8e4

KNN_THRESHOLD = 0.3
COS_EPS = 1e-8
NORM_EPS = 1e-12
ALU = mybir.AluOpType
AF = mybir.ActivationFunctionType
DR = mybir.MatmulPerfMode.DoubleRow


def build(n_batches: int = BPC):
    nc = bacc.Bacc("TRN2", debug=False, num_devices=N_CORES)
    XT = nc.dram_tensor("XT", [n_batches, D_IN, N], BF16, kind="ExternalInput")
    # X^T in fp8, pair-interleaved for DoubleRow: [b, k, p, i, n] with
    # d = k*256 + i*128 + p
    XT8 = nc.dram_tensor("XT8", [n_batches, KDR, 128, 2, N], FP8,
                         kind="ExternalInput")
    W1T = nc.dram_tensor("W1T", [D_IN, D_H], BF16, kind="ExternalInput")
    b1 = nc.dram_tensor("b1", [D_H], F32, kind="ExternalInput")
    W2T = nc.dram_tensor("W2T", [D_H, D_OUT], BF16, kind="ExternalInput")
    b2 = nc.dram_tensor("b2", [D_OUT], F32, kind="ExternalInput")
    Y = nc.dram_tensor("Y", [n_batches, N, D_OUT], F32, kind="ExternalOutput")
    with tile.TileContext(nc) as tc, ExitStack() as ctx:
        _body(ctx, tc, XT.ap(), XT8.ap(), W1T.ap(), b1.ap(), W2T.ap(), b2.ap(),
              Y.ap(), n_batches)
    nc.compile()
    return nc


def _bcast_p(ap: bass.AP, parts: int = 128) -> bass.AP:
    """Broadcast a DRAM AP across `parts` partitions (partition-stride 0)."""
    return bass.AP(tensor=ap.tensor, offset=ap.offset, ap=[[0, parts]] + list(ap.ap))


class _GraphState:
    """Per-graph SBUF tiles threaded between pipeline phases."""
    __slots__ = ("XTb", "XT8b", "Yb", "xt", "xt8", "at", "ys1", "ys2",
                 "h1t", "ssqv", "rc03", "nrep", "degv", "dv", "drep", "oall")


def _body(ctx, tc, XT, XT8, W1T, b1, W2T, b2, Y, n_batches):
    nc = tc.nc

    nb = n_batches
    singles = ctx.enter_context(tc.tile_pool(name="singles", bufs=1))
    sqj = ctx.enter_context(tc.tile_pool(name="sqj", bufs=4))
    xt8pool = ctx.enter_context(tc.tile_pool(name="xt8pool", bufs=nb))
    xtpool = ctx.enter_context(tc.tile_pool(name="xtpool", bufs=2))
    apool = ctx.enter_context(tc.tile_pool(name="apool", bufs=nb))
    bvec = ctx.enter_context(tc.tile_pool(name="bvec", bufs=2 * nb))
    y1pool = ctx.enter_context(tc.tile_pool(name="y1pool", bufs=nb * NT))
    h1pool = ctx.enter_context(tc.tile_pool(name="h1pool", bufs=nb * HC))
    y2pool = ctx.enter_context(tc.tile_pool(name="y2pool", bufs=nb * NT))
    rppool = ctx.enter_context(tc.tile_pool(name="rppool", bufs=4))
    tmppool = ctx.enter_context(tc.tile_pool(name="tmppool", bufs=4))
    h2pool = ctx.enter_context(tc.tile_pool(name="h2pool", bufs=8))
    opool = ctx.enter_context(tc.tile_pool(name="opool", bufs=2))
    psA = ctx.enter_context(tc.tile_pool(name="psA", bufs=4, space="PSUM"))
    psB = ctx.enter_context(tc.tile_pool(name="psB", bufs=4, space="PSUM"))
    dramp = ctx.enter_context(tc.tile_pool(name="dramp", bufs=nb, space="DRAM"))

    # ---- one-time constants (plain loads, no prep chains) -------------------
    ident = singles.tile([128, 128], BF16)
    make_identity(nc, ident)
    identf = singles.tile([128, 128], F32)
    make_identity(nc, identf)

    b1col = singles.tile([128, HC], F32)
    nc.sync.dma_start(out=b1col, in_=bass.AP(tensor=b1.tensor, offset=b1.offset,
                                             ap=[[1, 128], [128, HC]]))
    b2rep = singles.tile([128, D_OUT], F32)
    nc.sync.dma_start(out=b2rep, in_=_bcast_p(b2))

    w1t = singles.tile([128, DTI, D_H], BF16, tag="w1t")
    nc.sync.dma_start(out=w1t, in_=W1T.rearrange("(dt p) h -> p dt h", p=128))
    w2t = singles.tile([128, HC, D_OUT], BF16, tag="w2t")
    nc.sync.dma_start(out=w2t, in_=W2T.rearrange("(k p) do -> p k do", p=128))

    inv_t = 1.0 / KNN_THRESHOLD

    # ---- per-phase emitters -------------------------------------------------
    def phase_a(g: _GraphState):
        # fp8 DoubleRow-packed X^T (feeds the gram matmuls); one DMA per graph
        g.xt8 = xt8pool.tile([128, KDR, 2, N], FP8, tag="xt8")
        nc.sync.dma_start(out=g.xt8, in_=g.XT8b.rearrange("k p i n -> p k i n"))
        g.at = None
        g.ys1 = []
        g.ys2 = []
        g.h1t = []

    def phase_b1(g: _GraphState):
        # pre-pass: row norms from the gram diagonal blocks; then the
        # norm -> reciprocal chain and the DRAM-bounce broadcast of n_j.
        # Emitted for all graphs before any gram so the bounce round-trip
        # hides behind the other graphs' pre-passes on the PE.
        g.ssqv = bvec.tile([128, NT], F32, tag="ssqv")
        for it in range(NT):
            psd = psB.tile([128, D_OUT], F32, tag="psB", name="psd")
            blk = slice(it * 128, (it + 1) * 128)
            for k in range(KDR):
                nc.tensor.matmul(psd, lhsT=g.xt8[:, k, :, blk],
                                 rhs=g.xt8[:, k, :, blk],
                                 start=(k == 0), stop=(k == KDR - 1),
                                 perf_mode=DR)
            dj = sqj.tile([128, 128], BF16, tag="dj")
            nc.vector.scalar_tensor_tensor(
                out=dj, in0=psd, scalar=1.0, in1=identf,
                op0=ALU.bypass, op1=ALU.mult,
                accum_out=g.ssqv[:, it:it + 1])
        ncol = bvec.tile([128, NT], F32, tag="ncol")
        nc.scalar.sqrt(out=ncol, in_=g.ssqv)
        nclamp = bvec.tile([128, NT], F32, tag="nclamp")
        nc.vector.tensor_scalar_max(nclamp, ncol, COS_EPS)
        rcol = bvec.tile([128, NT], F32, tag="rcol")
        nc.vector.reciprocal(out=rcol, in_=nclamp)
        g.rc03 = bvec.tile([128, NT], F32, tag="rc03")
        nc.vector.tensor_scalar_mul(g.rc03, rcol, inv_t)

        # bounce ncol -> DRAM -> Nrep (n_j replicated over partitions, bf16).
        # The scratch is bf16 so both DMAs are cast-free (HWDGE-eligible).
        ncol16 = bvec.tile([128, NT], BF16, tag="ncol16")
        nc.scalar.activation(out=ncol16, in_=ncol, func=AF.Copy)
        nscr = dramp.tile([1, N], BF16, tag="nscr")
        nflat = nscr[0]
        nc.sync.dma_start(
            out=bass.AP(tensor=nflat.tensor, offset=nflat.offset,
                        ap=[[1, 128], [128, NT]]),
            in_=ncol16)
        g.nrep = rppool.tile([128, N], BF16, tag="nrep")
        nc.sync.dma_start(out=g.nrep, in_=_bcast_p(nflat))

    def phase_b2(g: _GraphState):
        # main pass: G row tiles -> threshold -> A (+ self loop), deg fused.
        # Threshold evictions alternate DVE / Pool to halve the per-engine
        # load; the self-loop add runs on DVE (bf16 sbuf, 2x mode).
        g.at = apool.tile([128, NT, N], BF16, tag="at")
        g.degv = bvec.tile([128, 2 * NT], F32, tag="degv")
        for it in range(NT):
            a_t = g.at[:, it]
            for jh in range(2):
                ps = psA.tile([128, 512], F32, tag="psA")
                for k in range(KDR):
                    nc.tensor.matmul(
                        ps, lhsT=g.xt8[:, k, :, it * 128:(it + 1) * 128],
                        rhs=g.xt8[:, k, :, jh * 512:(jh + 1) * 512],
                        start=(k == 0), stop=(k == KDR - 1), perf_mode=DR)
                nc.vector.scalar_tensor_tensor(
                    out=a_t[:, jh * 512:(jh + 1) * 512], in0=ps,
                    scalar=g.rc03[:, it:it + 1],
                    in1=g.nrep[:, jh * 512:(jh + 1) * 512],
                    op0=ALU.mult, op1=ALU.is_gt,
                    accum_out=g.degv[:, jh * NT + it:jh * NT + it + 1])
            nc.gpsimd.tensor_tensor(out=a_t[:, it * 128:(it + 1) * 128],
                                    in0=a_t[:, it * 128:(it + 1) * 128],
                                    in1=ident, op=ALU.add)

        # deg -> d = deg^-1/2 -> Drep bounce
        dsum = bvec.tile([128, NT], F32, tag="dsum")
        nc.vector.tensor_tensor(out=dsum, in0=g.degv[:, 0:NT],
                                in1=g.degv[:, NT:2 * NT], op=ALU.add)
        sqd = bvec.tile([128, NT], F32, tag="sqd")
        nc.scalar.activation(out=sqd, in_=dsum, func=AF.Sqrt, bias=1.0)
        g.dv = bvec.tile([128, NT], F32, tag="dv")
        nc.vector.reciprocal(out=g.dv, in_=sqd)

        dv16 = bvec.tile([128, NT], BF16, tag="dv16")
        nc.scalar.activation(out=dv16, in_=g.dv, func=AF.Copy)
        dscr = dramp.tile([1, N], BF16, tag="dscr")
        dflat = dscr[0]
        nc.sync.dma_start(
            out=bass.AP(tensor=dflat.tensor, offset=dflat.offset,
                        ap=[[1, 128], [128, NT]]),
            in_=dv16)
        g.drep = rppool.tile([128, N], BF16, tag="drep")
        nc.sync.dma_start(out=g.drep, in_=_bcast_p(dflat))

    def phase_c(g: _GraphState):
        # G1 = X @ W1.T [n, h]; evict scaled by d -> Ys1 bf16 (Act engine).
        # X^T bf16 is loaded JIT here (its only consumer) as one DMA.
        g.xt = xtpool.tile([128, DTI, N], BF16, tag="xt")
        nc.sync.dma_start(out=g.xt, in_=g.XTb.rearrange("(dt p) n -> p dt n",
                                                        p=128))
        for it in range(NT):
            ps = psB.tile([128, D_H], F32, tag="psB")
            for dt in range(DTI):
                nc.tensor.matmul(ps, lhsT=g.xt[:, dt, it * 128:(it + 1) * 128],
                                 rhs=w1t[:, dt], start=(dt == 0),
                                 stop=(dt == DTI - 1))
            y1 = y1pool.tile([128, D_H], BF16, tag="y1")
            nc.scalar.activation(out=y1, in_=ps, func=AF.Copy,
                                 scale=g.dv[:, it:it + 1])
            g.ys1.append(y1)

    def phase_d(g: _GraphState):
        # M1^T = (A diag(d) G1)^T over 4 concurrent PSUM groups (hc x ih),
        # K-contiguous in jt; H1^T = relu(d_i * M1^T + b1).
        # tmp-mult on Pool, relu on Act: no DVE involvement.
        for hc in range(HC):
            g.h1t.append(h1pool.tile([128, N], BF16, tag="h1", name="h1"))
        # hc-outer: each hc's 2-bank psum group evicts while the next hc
        # (or next graph) accumulates, so the PE never waits on eviction.
        for hc in range(HC):
            pss = [psA.tile([128, 512], F32, tag="psA", name="psd2")
                   for _ in range(2)]
            for jt in range(NT):
                st = jt == 0
                sp = jt == NT - 1
                lhsT = g.ys1[jt][:, hc * 128:(hc + 1) * 128]
                for ih in range(2):
                    nc.tensor.matmul(pss[ih], lhsT=lhsT,
                                     rhs=g.at[:, jt, ih * 512:(ih + 1) * 512],
                                     start=st, stop=sp)
            for ih in range(2):
                tmp = tmppool.tile([128, 512], F32, tag="tmp")
                nc.vector.tensor_tensor(out=tmp, in0=pss[ih],
                                        in1=g.drep[:, ih * 512:(ih + 1) * 512],
                                        op=ALU.mult)
                nc.scalar.activation(out=g.h1t[hc][:, ih * 512:(ih + 1) * 512],
                                     in_=tmp, func=AF.Relu,
                                     bias=b1col[:, hc:hc + 1])

    def phase_e(g: _GraphState):
        # Ys2 = d * (H1 @ W2.T), evicted on Pool.
        for it in range(NT):
            ps = psB.tile([128, D_OUT], F32, tag="psB")
            for hc in range(HC):
                nc.tensor.matmul(ps, lhsT=g.h1t[hc][:, it * 128:(it + 1) * 128],
                                 rhs=w2t[:, hc], start=(hc == 0),
                                 stop=(hc == HC - 1))
            y2 = y2pool.tile([128, D_OUT], BF16, tag="y2")
            nc.scalar.activation(out=y2, in_=ps, func=AF.Copy,
                                 scale=g.dv[:, it:it + 1])
            g.ys2.append(y2)

    def phase_f(g: _GraphState):
        # M2 = A Ys2; H2 = d*M2 + b2 (single Pool op); row-normalize; one
        # batched Y DMA per graph at the end.
        g.oall = opool.tile([128, NT, D_OUT], F32, tag="oall")
        for it in range(NT):
            ps = psB.tile([128, D_OUT], F32, tag="psB")
            for jt in range(NT):
                nc.tensor.matmul(ps, lhsT=g.at[:, jt, it * 128:(it + 1) * 128],
                                 rhs=g.ys2[jt], start=(jt == 0),
                                 stop=(jt == NT - 1))
            h2a = h2pool.tile([128, D_OUT], F32, tag="h2a")
            nc.scalar.activation(out=h2a, in_=ps, func=AF.Copy,
                                 scale=g.dv[:, it:it + 1])
            h2 = h2pool.tile([128, D_OUT], F32, tag="h2")
            nc.gpsimd.tensor_tensor(out=h2, in0=h2a, in1=b2rep, op=ALU.add)
            sj2 = sqj.tile([128, D_OUT], F32, tag="sqj2")
            ssq2 = bvec.tile([128, 1], F32, tag="ssq2")
            nc.scalar.activation(out=sj2, in_=h2, func=AF.Square,
                                 accum_out=ssq2)
            nrm2 = bvec.tile([128, 1], F32, tag="nrm2")
            nc.scalar.sqrt(out=nrm2, in_=ssq2)
            cl2 = bvec.tile([128, 1], F32, tag="cl2")
            nc.vector.tensor_scalar_max(cl2, nrm2, NORM_EPS)
            inv2 = bvec.tile([128, 1], F32, tag="inv2")
            nc.vector.reciprocal(out=inv2, in_=cl2)
            nc.scalar.activation(out=g.oall[:, it], in_=h2, func=AF.Copy,
                                 scale=inv2)
        nc.sync.dma_start(out=g.Yb.rearrange("(it p) do -> p it do", p=128),
                          in_=g.oall)

    # ---- wave-pipelined driver: all graphs in flight, phase-major -----------
    gs = []
    for bi in range(n_batches):
        g = _GraphState()
        g.XTb, g.XT8b, g.Yb = XT[bi], XT8[bi], Y[bi]
        gs.append(g)

    for g in gs:
        phase_a(g)
    for g in gs:
        phase_b1(g)
    for g in gs:
        phase_b2(g)
    for g in gs:
        phase_c(g)
    for g in gs:
        phase_d(g)
    for g in gs:
        phase_e(g)
    for g in gs:
        phase_f(g)


_NC_CACHE = {}


def _get_nc(n_batches: int = BPC):
    if n_batches not in _NC_CACHE:
        _NC_CACHE[n_batches] = build(n_batches)
    return _NC_CACHE[n_batches]


def make_in_maps(X, W1, b1, W2, b2, bpc: int = BPC):
    X = np.asarray(X, dtype=np.float32)
    nb = len(X)
    Xt = X.astype(ml_dtypes.bfloat16).transpose(0, 2, 1)   # [B, D, N] bf16
    XTb16 = np.ascontiguousarray(Xt)
    # DoubleRow pair-interleaved fp8: [b, k, p, i, n], d = k*256 + i*128 + p
    XT8 = np.ascontiguousarray(
        Xt.reshape(nb, KDR, 2, 128, N).transpose(0, 1, 3, 2, 4)
        .astype(ml_dtypes.float8_e4m3))
    W1T = np.ascontiguousarray(
        np.asarray(W1, dtype=np.float32).T.astype(ml_dtypes.bfloat16))
    W2T = np.ascontiguousarray(
        np.asarray(W2, dtype=np.float32).T.astype(ml_dtypes.bfloat16))
    b1 = np.ascontiguousarray(np.asarray(b1, dtype=np.float32))
    b2 = np.ascontiguousarray(np.asarray(b2, dtype=np.float32))
    return [
        {"XT": XTb16[c * bpc:(c + 1) * bpc], "XT8": XT8[c * bpc:(c + 1) * bpc],
         "W1T": W1T, "b1": b1, "W2T": W2T, "b2": b2}
        for c in range(nb // bpc)
    ]


def kernel(X, W1, b1, W2, b2):
    nc = _get_nc()
    in_maps = make_in_maps(X, W1, b1, W2, b2)
    res = run_bass_kernel_spmd(nc, in_maps, core_ids=list(range(N_CORES)))
    return np.concatenate([r["Y"] for r in res.results], axis=0)


# revision 19
# speedup vs baseline: 1.1949x; 1.1949x over previous
"""BatchedGCN Trainium2 kernel.

Per graph (batch element):
  norms_i = ||X_i||;  A = (X@X.T > 0.3*n_i*n_j) + I ; deg = rowsum(A); d = deg^-1/2
  H1 = relu(diag(d) A diag(d) (X @ W1.T) + b1)
  H2 = diag(d) A diag(d) (H1 @ W2.T) + b2
  out = H2 / max(||H2_row||, 1e-12)

Key implementation choices:
- The cosine threshold runs in un-normalized form:
  Xn_i . Xn_j > t  <=>  (X_i . X_j) * (1/max(n_i,eps)) / t > n_j.
  The diag(norm) factor relating X to Xn cancels against the un-normalized
  X used in the first linear layer, so the output path needs no norms.
- The gram matrix G = X X^T runs in fp8 (DoubleRow, 2x rate); the
  thresholding margin is ~40% of the bound while fp8 dot-product error is
  <0.5%, so A is bit-exact.  Row norms are read off G's diagonal blocks
  (computed in a cheap per-row-tile pre-pass), so they are fp8-accurate -
  again only used for the threshold bound.
- The two propagations and both linear layers run in bf16 with fp32 PSUM.
- Sharding: data-parallel over B=32 across 8 cores (4 graphs each),
  weights replicated.  Host-side layout prep ships X^T pre-cast (bf16 and
  DoubleRow-packed fp8) and transposed weights, so the kernel needs no
  on-chip transposes or casts.
- Engine balance: PSUM evictions are spread over DVE / Pool / Act so the
  tensor engine is the only near-saturated engine; all DMA descriptor
  generation is on the SP (sync) hardware DGE, none on Pool.
- Phase waves: every phase is emitted for all resident graphs before the
  next phase, so each graph's latency chains (norm -> DRAM-bounce
  broadcast -> threshold, psum eviction chains) hide behind the other
  graphs' dense matmuls and the PE stays continuously busy (p-state).
"""

from contextlib import ExitStack

import ml_dtypes
import numpy as np

import concourse.bass as bass
import concourse.mybir as mybir
import concourse.tile as tile
from concourse import bacc
from concourse.bass_utils import run_bass_kernel_spmd
from concourse.masks import make_identity

B, N, D_IN, D_H, D_OUT = 32, 1024, 768, 256, 128
N_CORES = 8
BPC = B // N_CORES          # graphs per core
NT = N // 128               # 8 row tiles
DTI = D_IN // 128           # 6 input-dim tiles
HC = D_H // 128             # 2 hidden chunks
KDR = D_IN // 256           # 3 DoubleRow K-chunks
F32 = mybir.dt.float32
BF16 = mybir.dt.bfloat16
FP8 = mybir.dt.float8e4

KNN_THRESHOLD = 0.3
COS_EPS = 1e-8
NORM_EPS = 1e-12
ALU = mybir.AluOpType
AF = mybir.ActivationFunctionType
DR = mybir.MatmulPerfMode.DoubleRow


def build(n_batches: int = BPC):
    nc = bacc.Bacc("TRN2", debug=False, num_devices=N_CORES)
    XT = nc.dram_tensor("XT", [n_batches, D_IN, N], BF16, kind="ExternalInput")
    # X^T in fp8, pair-interleaved for DoubleRow: [b, k, p, i, n] with
    # d = k*256 + i*128 + p
    XT8 = nc.dram_tensor("XT8", [n_batches, KDR, 128, 2, N], FP8,
                         kind="ExternalInput")
    W1T = nc.dram_tensor("W1T", [D_IN, D_H], BF16, kind="ExternalInput")
    b1 = nc.dram_tensor("b1", [D_H], F32, kind="ExternalInput")
    W2T = nc.dram_tensor("W2T", [D_H, D_OUT], BF16, kind="ExternalInput")
    b2 = nc.dram_tensor("b2", [D_OUT], F32, kind="ExternalInput")
    Y = nc.dram_tensor("Y", [n_batches, N, D_OUT], F32, kind="ExternalOutput")
    with tile.TileContext(nc) as tc, ExitStack() as ctx:
        _body(ctx, tc, XT.ap(), XT8.ap(), W1T.ap(), b1.ap(), W2T.ap(), b2.ap(),
              Y.ap(), n_batches)
    nc.compile()
    return nc


def _bcast_p(ap: bass.AP, parts: int = 128) -> bass.AP:
    """Broadcast a DRAM AP across `parts` partitions (partition-stride 0)."""
    return bass.AP(tensor=ap.tensor, offset=ap.offset, ap=[[0, parts]] + list(ap.ap))


class _GraphState:
    """Per-graph SBUF tiles threaded between pipeline phases."""
    __slots__ = ("XTb", "XT8b", "Yb", "xt", "xt8", "at", "ys1", "ys2",
                 "h1t", "ssqv", "rc03", "nrep", "degv", "dv", "drep", "oall")


def _body(ctx, tc, XT, XT8, W1T, b1, W2T, b2, Y, n_batches):
    nc = tc.nc

    nb = n_batches
    singles = ctx.enter_context(tc.tile_pool(name="singles", bufs=1))
    sqj = ctx.enter_context(tc.tile_pool(name="sqj", bufs=4))
    xt8pool = ctx.enter_context(tc.tile_pool(name="xt8pool", bufs=nb))
    xtpool = ctx.enter_context(tc.tile_pool(name="xtpool", bufs=2))
    apool = ctx.enter_context(tc.tile_pool(name="apool", bufs=nb))
    bvec = ctx.enter_context(tc.tile_pool(name="bvec", bufs=2 * nb))
    y1pool = ctx.enter_context(tc.tile_pool(name="y1pool", bufs=nb * NT))
    h1pool = ctx.enter_context(tc.tile_pool(name="h1pool", bufs=nb * HC))
    y2pool = ctx.enter_context(tc.tile_pool(name="y2pool", bufs=nb * NT))
    rppool = ctx.enter_context(tc.tile_pool(name="rppool", bufs=4))
    tmppool = ctx.enter_context(tc.tile_pool(name="tmppool", bufs=4))
    h2pool = ctx.enter_context(tc.tile_pool(name="h2pool", bufs=8))
    opool = ctx.enter_context(tc.tile_pool(name="opool", bufs=2))
    psA = ctx.enter_context(tc.tile_pool(name="psA", bufs=4, space="PSUM"))
    psB = ctx.enter_context(tc.tile_pool(name="psB", bufs=4, space="PSUM"))
    dramp = ctx.enter_context(tc.tile_pool(name="dramp", bufs=nb, space="DRAM"))

    # ---- one-time constants (plain loads, no prep chains) -------------------
    ident = singles.tile([128, 128], BF16)
    make_identity(nc, ident)
    identf = singles.tile([128, 128], F32)
    make_identity(nc, identf)

    b1col = singles.tile([128, HC], F32)
    nc.sync.dma_start(out=b1col, in_=bass.AP(tensor=b1.tensor, offset=b1.offset,
                                             ap=[[1, 128], [128, HC]]))
    b2rep = singles.tile([128, D_OUT], F32)
    nc.sync.dma_start(out=b2rep, in_=_bcast_p(b2))

    w1t = singles.tile([128, DTI, D_H], BF16, tag="w1t")
    nc.sync.dma_start(out=w1t, in_=W1T.rearrange("(dt p) h -> p dt h", p=128))
    w2t = singles.tile([128, HC, D_OUT], BF16, tag="w2t")
    nc.sync.dma_start(out=w2t, in_=W2T.rearrange("(k p) do -> p k do", p=128))

    inv_t = 1.0 / KNN_THRESHOLD

    # ---- per-phase emitters -------------------------------------------------
    def phase_a(g: _GraphState):
        # All input loads issued upfront on the sync HWDGE queue, which
        # carries no compute-dependent waits (bounces live on gpsimd), so
        # transfers stream from t=0.
        g.xt8 = xt8pool.tile([128, KDR, 2, N], FP8, tag="xt8")
        nc.sync.dma_start(out=g.xt8, in_=g.XT8b.rearrange("k p i n -> p k i n"))
        g.xt = xtpool.tile([128, DTI, N], BF16, tag="xt")
        nc.sync.dma_start(out=g.xt, in_=g.XTb.rearrange("(dt p) n -> p dt n",
                                                        p=128))
        g.at = None
        g.ys1 = []
        g.ys2 = []
        g.h1t = []

    def phase_b1(g: _GraphState):
        # pre-pass: row norms from the gram diagonal blocks; then the
        # norm -> reciprocal chain and the DRAM-bounce broadcast of n_j.
        # Emitted for all graphs before any gram so the bounce round-trip
        # hides behind the other graphs' pre-passes on the PE.
        g.ssqv = bvec.tile([128, NT], F32, tag="ssqv")
        for it in range(NT):
            psd = psB.tile([128, D_OUT], F32, tag="psB", name="psd")
            blk = slice(it * 128, (it + 1) * 128)
            for k in range(KDR):
                nc.tensor.matmul(psd, lhsT=g.xt8[:, k, :, blk],
                                 rhs=g.xt8[:, k, :, blk],
                                 start=(k == 0), stop=(k == KDR - 1),
                                 perf_mode=DR)
            dj = sqj.tile([128, 128], BF16, tag="dj")
            nc.vector.scalar_tensor_tensor(
                out=dj, in0=psd, scalar=1.0, in1=identf,
                op0=ALU.bypass, op1=ALU.mult,
                accum_out=g.ssqv[:, it:it + 1])
        ncol = bvec.tile([128, NT], F32, tag="ncol")
        nc.scalar.sqrt(out=ncol, in_=g.ssqv)
        nclamp = bvec.tile([128, NT], F32, tag="nclamp")
        nc.vector.tensor_scalar_max(nclamp, ncol, COS_EPS)
        rcol = bvec.tile([128, NT], F32, tag="rcol")
        nc.vector.reciprocal(out=rcol, in_=nclamp)
        g.rc03 = bvec.tile([128, NT], F32, tag="rc03")
        nc.vector.tensor_scalar_mul(g.rc03, rcol, inv_t)

        # bounce ncol -> DRAM -> Nrep (n_j replicated over partitions, bf16).
        # On gpsimd: keeps compute-dependent waits off the input DMA queue,
        # and gpsimd DMAs may cast f32->bf16 on the fly.
        nscr = dramp.tile([1, N], F32, tag="nscr")
        nflat = nscr[0]
        nc.gpsimd.dma_start(
            out=bass.AP(tensor=nflat.tensor, offset=nflat.offset,
                        ap=[[1, 128], [128, NT]]),
            in_=ncol)
        g.nrep = rppool.tile([128, N], BF16, tag="nrep")
        nc.gpsimd.dma_start(out=g.nrep, in_=_bcast_p(nflat))

    def phase_b2(g: _GraphState):
        # main pass: G row tiles -> threshold -> A (+ self loop), deg fused.
        # Threshold evictions alternate DVE / Pool to halve the per-engine
        # load; the self-loop add runs on DVE (bf16 sbuf, 2x mode).
        g.at = apool.tile([128, NT, N], BF16, tag="at")
        g.degv = bvec.tile([128, 2 * NT], F32, tag="degv")
        for it in range(NT):
            a_t = g.at[:, it]
            for jh in range(2):
                ps = psA.tile([128, 512], F32, tag="psA")
                for k in range(KDR):
                    nc.tensor.matmul(
                        ps, lhsT=g.xt8[:, k, :, it * 128:(it + 1) * 128],
                        rhs=g.xt8[:, k, :, jh * 512:(jh + 1) * 512],
                        start=(k == 0), stop=(k == KDR - 1), perf_mode=DR)
                nc.vector.scalar_tensor_tensor(
                    out=a_t[:, jh * 512:(jh + 1) * 512], in0=ps,
                    scalar=g.rc03[:, it:it + 1],
                    in1=g.nrep[:, jh * 512:(jh + 1) * 512],
                    op0=ALU.mult, op1=ALU.is_gt,
                    accum_out=g.degv[:, jh * NT + it:jh * NT + it + 1])
            nc.gpsimd.tensor_tensor(out=a_t[:, it * 128:(it + 1) * 128],
                                    in0=a_t[:, it * 128:(it + 1) * 128],
                                    in1=ident, op=ALU.add)

        # deg -> d = deg^-1/2 -> Drep bounce
        dsum = bvec.tile([128, NT], F32, tag="dsum")
        nc.vector.tensor_tensor(out=dsum, in0=g.degv[:, 0:NT],
                                in1=g.degv[:, NT:2 * NT], op=ALU.add)
        sqd = bvec.tile([128, NT], F32, tag="sqd")
        nc.scalar.activation(out=sqd, in_=dsum, func=AF.Sqrt, bias=1.0)
        g.dv = bvec.tile([128, NT], F32, tag="dv")
        nc.vector.reciprocal(out=g.dv, in_=sqd)

        dscr = dramp.tile([1, N], F32, tag="dscr")
        dflat = dscr[0]
        nc.gpsimd.dma_start(
            out=bass.AP(tensor=dflat.tensor, offset=dflat.offset,
                        ap=[[1, 128], [128, NT]]),
            in_=g.dv)
        g.drep = rppool.tile([128, N], BF16, tag="drep")
        nc.gpsimd.dma_start(out=g.drep, in_=_bcast_p(dflat))

    def phase_c(g: _GraphState):
        # G1 = X @ W1.T [n, h]; evict scaled by d -> Ys1 bf16 (Act engine).
        for it in range(NT):
            ps = psB.tile([128, D_H], F32, tag="psB")
            for dt in range(DTI):
                nc.tensor.matmul(ps, lhsT=g.xt[:, dt, it * 128:(it + 1) * 128],
                                 rhs=w1t[:, dt], start=(dt == 0),
                                 stop=(dt == DTI - 1))
            y1 = y1pool.tile([128, D_H], BF16, tag="y1")
            nc.scalar.activation(out=y1, in_=ps, func=AF.Copy,
                                 scale=g.dv[:, it:it + 1])
            g.ys1.append(y1)

    def phase_d(g: _GraphState):
        # M1^T = (A diag(d) G1)^T over 4 concurrent PSUM groups (hc x ih),
        # K-contiguous in jt; H1^T = relu(d_i * M1^T + b1).
        # tmp-mult on Pool, relu on Act: no DVE involvement.
        for hc in range(HC):
            g.h1t.append(h1pool.tile([128, N], BF16, tag="h1", name="h1"))
        # hc-outer: each hc's 2-bank psum group evicts while the next hc
        # (or next graph) accumulates, so the PE never waits on eviction.
        for hc in range(HC):
            pss = [psA.tile([128, 512], F32, tag="psA", name="psd2")
                   for _ in range(2)]
            for jt in range(NT):
                st = jt == 0
                sp = jt == NT - 1
                lhsT = g.ys1[jt][:, hc * 128:(hc + 1) * 128]
                for ih in range(2):
                    nc.tensor.matmul(pss[ih], lhsT=lhsT,
                                     rhs=g.at[:, jt, ih * 512:(ih + 1) * 512],
                                     start=st, stop=sp)
            for ih in range(2):
                tmp = tmppool.tile([128, 512], F32, tag="tmp")
                nc.vector.tensor_tensor(out=tmp, in0=pss[ih],
                                        in1=g.drep[:, ih * 512:(ih + 1) * 512],
                                        op=ALU.mult)
                nc.scalar.activation(out=g.h1t[hc][:, ih * 512:(ih + 1) * 512],
                                     in_=tmp, func=AF.Relu,
                                     bias=b1col[:, hc:hc + 1])

    def phase_e(g: _GraphState):
        # Ys2 = d * (H1 @ W2.T), evicted on Pool.
        for it in range(NT):
            ps = psB.tile([128, D_OUT], F32, tag="psB")
            for hc in range(HC):
                nc.tensor.matmul(ps, lhsT=g.h1t[hc][:, it * 128:(it + 1) * 128],
                                 rhs=w2t[:, hc], start=(hc == 0),
                                 stop=(hc == HC - 1))
            y2 = y2pool.tile([128, D_OUT], BF16, tag="y2")
            if it % 2 == 0:
                nc.scalar.activation(out=y2, in_=ps, func=AF.Copy,
                                     scale=g.dv[:, it:it + 1])
            else:
                nc.vector.tensor_scalar(out=y2, in0=ps,
                                        scalar1=g.dv[:, it:it + 1],
                                        scalar2=None, op0=ALU.mult)
            g.ys2.append(y2)

    def phase_f(g: _GraphState):
        # M2 = A Ys2; H2 = d*M2 + b2 (single Pool op); row-normalize; one
        # batched Y DMA per graph at the end.
        g.oall = opool.tile([128, NT, D_OUT], F32, tag="oall")
        for it in range(NT):
            ps = psB.tile([128, D_OUT], F32, tag="psB")
            for jt in range(NT):
                nc.tensor.matmul(ps, lhsT=g.at[:, jt, it * 128:(it + 1) * 128],
                                 rhs=g.ys2[jt], start=(jt == 0),
                                 stop=(jt == NT - 1))
            h2a = h2pool.tile([128, D_OUT], F32, tag="h2a")
            nc.scalar.activation(out=h2a, in_=ps, func=AF.Copy,
                                 scale=g.dv[:, it:it + 1])
            h2 = h2pool.tile([128, D_OUT], F32, tag="h2")
            nc.gpsimd.tensor_tensor(out=h2, in0=h2a, in1=b2rep, op=ALU.add)
            sj2 = sqj.tile([128, D_OUT], F32, tag="sqj2")
            ssq2 = bvec.tile([128, 1], F32, tag="ssq2")
            nc.vector.scalar_tensor_tensor(out=sj2, in0=h2, scalar=1.0,
                                           in1=h2, op0=ALU.bypass,
                                           op1=ALU.mult, accum_out=ssq2)
            nrm2 = bvec.tile([128, 1], F32, tag="nrm2")
            nc.scalar.sqrt(out=nrm2, in_=ssq2)
            cl2 = bvec.tile([128, 1], F32, tag="cl2")
            nc.vector.tensor_scalar_max(cl2, nrm2, NORM_EPS)
            inv2 = bvec.tile([128, 1], F32, tag="inv2")
            nc.vector.reciprocal(out=inv2, in_=cl2)
            nc.vector.tensor_scalar(out=g.oall[:, it], in0=h2, scalar1=inv2,
                                    scalar2=None, op0=ALU.mult)
        nc.sync.dma_start(out=g.Yb.rearrange("(it p) do -> p it do", p=128),
                          in_=g.oall)

    # ---- wave-pipelined driver: all graphs in flight, phase-major -----------
    gs = []
    for bi in range(n_batches):
        g = _GraphState()
        g.XTb, g.XT8b, g.Yb = XT[bi], XT8[bi], Y[bi]
        gs.append(g)

    for g in gs:
        phase_a(g)
    for g in gs:
        phase_b1(g)
    # offset interleave: graph i's gram/threshold overlaps graph i-1's
    # first linear layer on the PE while DVE digests the thresholds.
    for i in range(n_batches):
        phase_b2(gs[i])
        if i >= 1:
            phase_c(gs[i - 1])
    phase_c(gs[n_batches - 1])
    for g in gs:
        phase_d(g)
    for g in gs:
        phase_e(g)
        phase_f(g)


_NC_CACHE = {}


def _get_nc(n_batches: int = BPC):
    if n_batches not in _NC_CACHE:
        _NC_CACHE[n_batches] = build(n_batches)
    return _NC_CACHE[n_batches]


def make_in_maps(X, W1, b1, W2, b2, bpc: int = BPC):
    X = np.asarray(X, dtype=np.float32)
    nb = len(X)
    Xt = X.astype(ml_dtypes.bfloat16).transpose(0, 2, 1)   # [B, D, N] bf16
    XTb16 = np.ascontiguousarray(Xt)
    # DoubleRow pair-interleaved fp8: [b, k, p, i, n], d = k*256 + i*128 + p
    XT8 = np.ascontiguousarray(
        Xt.reshape(nb, KDR, 2, 128, N).transpose(0, 1, 3, 2, 4)
        .astype(ml_dtypes.float8_e4m3))
    W1T = np.ascontiguousarray(
        np.asarray(W1, dtype=np.float32).T.astype(ml_dtypes.bfloat16))
    W2T = np.ascontiguousarray(
        np.asarray(W2, dtype=np.float32).T.astype(ml_dtypes.bfloat16))
    b1 = np.ascontiguousarray(np.asarray(b1, dtype=np.float32))
    b2 = np.ascontiguousarray(np.asarray(b2, dtype=np.float32))
    return [
        {"XT": XTb16[c * bpc:(c + 1) * bpc], "XT8": XT8[c * bpc:(c + 1) * bpc],
         "W1T": W1T, "b1": b1, "W2T": W2T, "b2": b2}
        for c in range(nb // bpc)
    ]


def kernel(X, W1, b1, W2, b2):
    nc = _get_nc()
    in_maps = make_in_maps(X, W1, b1, W2, b2)
    res = run_bass_kernel_spmd(nc, in_maps, core_ids=list(range(N_CORES)))
    return np.concatenate([r["Y"] for r in res.results], axis=0)


# revision 20
# speedup vs baseline: 1.2156x; 1.0173x over previous
"""BatchedGCN Trainium2 kernel.

Per graph (batch element):
  norms_i = ||X_i||;  A = (X@X.T > 0.3*n_i*n_j) + I ; deg = rowsum(A); d = deg^-1/2
  H1 = relu(diag(d) A diag(d) (X @ W1.T) + b1)
  H2 = diag(d) A diag(d) (H1 @ W2.T) + b2
  out = H2 / max(||H2_row||, 1e-12)

Key implementation choices:
- The cosine threshold runs in un-normalized form:
  Xn_i . Xn_j > t  <=>  (X_i . X_j) * (1/max(n_i,eps)) / t > n_j.
  The diag(norm) factor relating X to Xn cancels against the un-normalized
  X used in the first linear layer, so the output path needs no norms.
- The gram matrix G = X X^T runs in fp8 (DoubleRow, 2x rate); the
  thresholding margin is ~40% of the bound while fp8 dot-product error is
  <0.5%, so A is bit-exact.  Row norms are read off G's diagonal blocks
  (computed in a cheap per-row-tile pre-pass), so they are fp8-accurate -
  again only used for the threshold bound.
- The two propagations and both linear layers run in bf16 with fp32 PSUM.
- Bias folding: the layer biases enter the propagation PSUMs as rank-1
  K=1 matmuls b/d_i (using a bounced 1/d row), so each propagation tile
  needs exactly ONE eviction op.  relu(d*u) = d*relu(u) moves the outer
  degree scale of layer 1 into layer 2's eviction scalar (d^2).
- Engine balance: PSUM evictions are only legal on DVE/Act; A-matrix
  self-loop adds run on Pool; all input DMA descriptor generation is on
  the sync HWDGE queue (no compute waits), bounces on gpsimd.
- Phase waves with a b2/c offset interleave keep each graph's latency
  chains (norm -> DRAM-bounce broadcast -> threshold) hidden behind the
  other graphs' dense matmuls, so the PE stays continuously busy.
"""

from contextlib import ExitStack

import ml_dtypes
import numpy as np

import concourse.bass as bass
import concourse.mybir as mybir
import concourse.tile as tile
from concourse import bacc
from concourse.bass_utils import run_bass_kernel_spmd
from concourse.masks import make_identity

B, N, D_IN, D_H, D_OUT = 32, 1024, 768, 256, 128
N_CORES = 8
BPC = B // N_CORES          # graphs per core
NT = N // 128               # 8 row tiles
DTI = D_IN // 128           # 6 input-dim tiles
HC = D_H // 128             # 2 hidden chunks
KDR = D_IN // 256           # 3 DoubleRow K-chunks
F32 = mybir.dt.float32
BF16 = mybir.dt.bfloat16
FP8 = mybir.dt.float8e4

KNN_THRESHOLD = 0.3
COS_EPS = 1e-8
NORM_EPS = 1e-12
ALU = mybir.AluOpType
AF = mybir.ActivationFunctionType
DR = mybir.MatmulPerfMode.DoubleRow

N_RAW = 2                   # trailing row-tiles per graph thresholded from
                            # a bf16 SBUF copy instead of PSUM (Act+DVE-2x)


def build(n_batches: int = BPC):
    nc = bacc.Bacc("TRN2", debug=False, num_devices=N_CORES)
    XT = nc.dram_tensor("XT", [n_batches, D_IN, N], BF16, kind="ExternalInput")
    # X^T in fp8, pair-interleaved for DoubleRow: [b, k, p, i, n] with
    # d = k*256 + i*128 + p
    XT8 = nc.dram_tensor("XT8", [n_batches, KDR, 128, 2, N], FP8,
                         kind="ExternalInput")
    W1T = nc.dram_tensor("W1T", [D_IN, D_H], BF16, kind="ExternalInput")
    b1 = nc.dram_tensor("b1", [D_H], F32, kind="ExternalInput")
    W2T = nc.dram_tensor("W2T", [D_H, D_OUT], BF16, kind="ExternalInput")
    b2 = nc.dram_tensor("b2", [D_OUT], F32, kind="ExternalInput")
    Y = nc.dram_tensor("Y", [n_batches, N, D_OUT], F32, kind="ExternalOutput")
    with tile.TileContext(nc) as tc, ExitStack() as ctx:
        _body(ctx, tc, XT.ap(), XT8.ap(), W1T.ap(), b1.ap(), W2T.ap(), b2.ap(),
              Y.ap(), n_batches)
    nc.compile()
    return nc


def _bcast_p(ap: bass.AP, parts: int = 128) -> bass.AP:
    """Broadcast a DRAM AP across `parts` partitions (partition-stride 0)."""
    return bass.AP(tensor=ap.tensor, offset=ap.offset, ap=[[0, parts]] + list(ap.ap))


def _row1(ap: bass.AP, n: int) -> bass.AP:
    """View a flat DRAM AP as a [1, n] row."""
    return bass.AP(tensor=ap.tensor, offset=ap.offset, ap=[[0, 1], [1, n]])


class _GraphState:
    """Per-graph SBUF tiles threaded between pipeline phases."""
    __slots__ = ("XTb", "XT8b", "Yb", "xt", "xt8", "at", "ys1", "ys2",
                 "h1t", "ssqv", "rc03", "nrep", "degv", "dv", "dvsq",
                 "invd", "oall", "h2s", "ssqall")


def _body(ctx, tc, XT, XT8, W1T, b1, W2T, b2, Y, n_batches):
    nc = tc.nc

    nb = n_batches
    singles = ctx.enter_context(tc.tile_pool(name="singles", bufs=1))
    sqj = ctx.enter_context(tc.tile_pool(name="sqj", bufs=4))
    xt8pool = ctx.enter_context(tc.tile_pool(name="xt8pool", bufs=nb))
    xtpool = ctx.enter_context(tc.tile_pool(name="xtpool", bufs=2))
    apool = ctx.enter_context(tc.tile_pool(name="apool", bufs=nb))
    bvec = ctx.enter_context(tc.tile_pool(name="bvec", bufs=2 * nb))
    y1pool = ctx.enter_context(tc.tile_pool(name="y1pool", bufs=nb * NT))
    h1pool = ctx.enter_context(tc.tile_pool(name="h1pool", bufs=nb * HC))
    y2pool = ctx.enter_context(tc.tile_pool(name="y2pool", bufs=nb * NT))
    rppool = ctx.enter_context(tc.tile_pool(name="rppool", bufs=4))
    h2pool = ctx.enter_context(tc.tile_pool(name="h2pool", bufs=NT + 4))
    opool = ctx.enter_context(tc.tile_pool(name="opool", bufs=2))
    psA = ctx.enter_context(tc.tile_pool(name="psA", bufs=2, space="PSUM"))
    psB = ctx.enter_context(tc.tile_pool(name="psB", bufs=4, space="PSUM"))
    dramp = ctx.enter_context(tc.tile_pool(name="dramp", bufs=nb, space="DRAM"))

    # ---- one-time constants (plain loads, no prep chains) -------------------
    ident = singles.tile([128, 128], BF16)
    make_identity(nc, ident)
    identf = singles.tile([128, 128], F32)
    make_identity(nc, identf)

    # bias rows in partition 0 (bf16, gpsimd DMA casts f32 -> bf16)
    b1row = singles.tile([1, D_H], BF16, tag="b1row")
    nc.gpsimd.dma_start(out=b1row, in_=_row1(b1, D_H))
    b2row = singles.tile([1, D_OUT], BF16, tag="b2row")
    nc.gpsimd.dma_start(out=b2row, in_=_row1(b2, D_OUT))

    w1t = singles.tile([128, DTI, D_H], BF16, tag="w1t")
    nc.sync.dma_start(out=w1t, in_=W1T.rearrange("(dt p) h -> p dt h", p=128))
    w2t = singles.tile([128, HC, D_OUT], BF16, tag="w2t")
    nc.sync.dma_start(out=w2t, in_=W2T.rearrange("(k p) do -> p k do", p=128))

    inv_t = 1.0 / KNN_THRESHOLD

    # ---- per-phase emitters -------------------------------------------------
    def phase_a(g: _GraphState):
        # All input loads issued upfront on the sync HWDGE queue, which
        # carries no compute-dependent waits (bounces live on gpsimd), so
        # transfers stream from t=0.
        g.xt8 = xt8pool.tile([128, KDR, 2, N], FP8, tag="xt8")
        nc.sync.dma_start(out=g.xt8, in_=g.XT8b.rearrange("k p i n -> p k i n"))
        g.xt = xtpool.tile([128, DTI, N], BF16, tag="xt")
        nc.sync.dma_start(out=g.xt, in_=g.XTb.rearrange("(dt p) n -> p dt n",
                                                        p=128))
        g.at = None
        g.ys1 = []
        g.ys2 = []
        g.h1t = []
        g.h2s = []

    def phase_b1(g: _GraphState):
        # pre-pass: row norms from the gram diagonal blocks; then the
        # norm -> reciprocal chain and the DRAM-bounce broadcast of n_j.
        # Emitted for all graphs before any gram so the bounce round-trip
        # hides behind the other graphs' pre-passes on the PE.
        g.ssqv = bvec.tile([128, NT], F32, tag="ssqv")
        for it in range(NT):
            psd = psB.tile([128, D_OUT], F32, tag="psB", name="psd")
            blk = slice(it * 128, (it + 1) * 128)
            for k in range(KDR):
                nc.tensor.matmul(psd, lhsT=g.xt8[:, k, :, blk],
                                 rhs=g.xt8[:, k, :, blk],
                                 start=(k == 0), stop=(k == KDR - 1),
                                 perf_mode=DR)
            dj = sqj.tile([128, 128], BF16, tag="dj")
            nc.vector.scalar_tensor_tensor(
                out=dj, in0=psd, scalar=1.0, in1=identf,
                op0=ALU.bypass, op1=ALU.mult,
                accum_out=g.ssqv[:, it:it + 1])
        ncol = bvec.tile([128, NT], F32, tag="ncol")
        nc.scalar.sqrt(out=ncol, in_=g.ssqv)
        nclamp = bvec.tile([128, NT], F32, tag="nclamp")
        nc.vector.tensor_scalar_max(nclamp, ncol, COS_EPS)
        rcol = bvec.tile([128, NT], F32, tag="rcol")
        nc.vector.reciprocal(out=rcol, in_=nclamp)
        g.rc03 = bvec.tile([128, NT], F32, tag="rc03")
        nc.vector.tensor_scalar_mul(g.rc03, rcol, inv_t)

        # bounce ncol -> DRAM -> Nrep (n_j replicated over partitions, bf16).
        # On gpsimd: keeps compute-dependent waits off the input DMA queue,
        # and gpsimd DMAs may cast f32->bf16 on the fly.
        nscr = dramp.tile([1, N], F32, tag="nscr")
        nflat = nscr[0]
        nc.gpsimd.dma_start(
            out=bass.AP(tensor=nflat.tensor, offset=nflat.offset,
                        ap=[[1, 128], [128, NT]]),
            in_=ncol)
        g.nrep = rppool.tile([128, N], BF16, tag="nrep")
        nc.gpsimd.dma_start(out=g.nrep, in_=_bcast_p(nflat))

    def phase_b2(g: _GraphState):
        # main pass: paired two-bank G row PSUMs -> one threshold eviction
        # per row tile -> A (+ self loop on Pool), deg row-sums fused.
        # The last N_RAW tiles are raw-copied bf16 by Act and thresholded
        # from SBUF by DVE (all-2-byte op).
        g.at = apool.tile([128, NT, N], BF16, tag="at")
        g.degv = bvec.tile([128, NT], F32, tag="degv")
        for it in range(NT):
            a_t = g.at[:, it]
            ps = psA.tile([128, N], F32, tag="psA")
            for jh in range(2):
                psh = ps[:, jh * 512:(jh + 1) * 512]
                for k in range(KDR):
                    nc.tensor.matmul(
                        psh, lhsT=g.xt8[:, k, :, it * 128:(it + 1) * 128],
                        rhs=g.xt8[:, k, :, jh * 512:(jh + 1) * 512],
                        start=(k == 0), stop=(k == KDR - 1), perf_mode=DR)
            if it < NT - N_RAW:
                nc.vector.scalar_tensor_tensor(
                    out=a_t, in0=ps, scalar=g.rc03[:, it:it + 1], in1=g.nrep,
                    op0=ALU.mult, op1=ALU.is_gt,
                    accum_out=g.degv[:, it:it + 1])
            else:
                raw = sqj.tile([128, N], BF16, tag="raw")
                nc.scalar.activation(out=raw, in_=ps, func=AF.Copy)
                nc.vector.scalar_tensor_tensor(
                    out=a_t, in0=raw, scalar=g.rc03[:, it:it + 1], in1=g.nrep,
                    op0=ALU.mult, op1=ALU.is_gt,
                    accum_out=g.degv[:, it:it + 1])
            nc.gpsimd.tensor_tensor(out=a_t[:, it * 128:(it + 1) * 128],
                                    in0=a_t[:, it * 128:(it + 1) * 128],
                                    in1=ident, op=ALU.add)

        # deg -> d = deg^-1/2 (and 1/d, d^2); bounce 1/d to a [1,N] row for
        # the rank-1 bias folds of both layers.
        sqd = bvec.tile([128, NT], F32, tag="sqd")
        nc.scalar.activation(out=sqd, in_=g.degv, func=AF.Sqrt, bias=1.0)
        g.dv = bvec.tile([128, NT], F32, tag="dv")
        nc.vector.reciprocal(out=g.dv, in_=sqd)
        g.dvsq = bvec.tile([128, NT], F32, tag="dvsq")
        nc.vector.tensor_tensor(out=g.dvsq, in0=g.dv, in1=g.dv, op=ALU.mult)

        iscr = dramp.tile([1, N], F32, tag="iscr")
        iflat = iscr[0]
        nc.gpsimd.dma_start(
            out=bass.AP(tensor=iflat.tensor, offset=iflat.offset,
                        ap=[[1, 128], [128, NT]]),
            in_=sqd)
        g.invd = rppool.tile([1, N], BF16, tag="invd")
        nc.gpsimd.dma_start(out=g.invd, in_=_row1(iflat, N))

    def phase_c(g: _GraphState):
        # G1 = X @ W1.T [n, h]; evict scaled by d -> Ys1 bf16 (Act engine).
        for it in range(NT):
            ps = psB.tile([128, D_H], F32, tag="psB")
            for dt in range(DTI):
                nc.tensor.matmul(ps, lhsT=g.xt[:, dt, it * 128:(it + 1) * 128],
                                 rhs=w1t[:, dt], start=(dt == 0),
                                 stop=(dt == DTI - 1))
            y1 = y1pool.tile([128, D_H], BF16, tag="y1")
            nc.scalar.activation(out=y1, in_=ps, func=AF.Copy,
                                 scale=g.dv[:, it:it + 1])
            g.ys1.append(y1)

    def phase_d(g: _GraphState):
        # M1^T + b1/d_i = (A diag(d) G1)^T + outer(b1, 1/d) over paired
        # two-bank PSUMs (one per hc); H1'^T = relu(psum) in ONE Act op.
        # The missing outer diag(d) resurfaces as d^2 in phase_e's scalar.
        for hc in range(HC):
            g.h1t.append(h1pool.tile([128, N], BF16, tag="h1", name="h1"))
        for hc in range(HC):
            pss = psA.tile([128, N], F32, tag="psA", name="psd2")
            for ih in range(2):
                nc.tensor.matmul(pss[:, ih * 512:(ih + 1) * 512],
                                 lhsT=b1row[:, hc * 128:(hc + 1) * 128],
                                 rhs=g.invd[:, ih * 512:(ih + 1) * 512],
                                 start=True, stop=False)
            for jt in range(NT):
                sp = jt == NT - 1
                lhsT = g.ys1[jt][:, hc * 128:(hc + 1) * 128]
                for ih in range(2):
                    nc.tensor.matmul(pss[:, ih * 512:(ih + 1) * 512],
                                     lhsT=lhsT,
                                     rhs=g.at[:, jt, ih * 512:(ih + 1) * 512],
                                     start=False, stop=sp)
            nc.scalar.activation(out=g.h1t[hc], in_=pss, func=AF.Relu)

    def phase_e(g: _GraphState):
        # Ys2 = d^2 * (H1' @ W2.T)  (the d^2 restores both diag(d) factors)
        for it in range(NT):
            ps = psB.tile([128, D_OUT], F32, tag="psB")
            for hc in range(HC):
                nc.tensor.matmul(ps, lhsT=g.h1t[hc][:, it * 128:(it + 1) * 128],
                                 rhs=w2t[:, hc], start=(hc == 0),
                                 stop=(hc == HC - 1))
            y2 = y2pool.tile([128, D_OUT], BF16, tag="y2")
            if it % 2 == 0:
                nc.scalar.activation(out=y2, in_=ps, func=AF.Copy,
                                     scale=g.dvsq[:, it:it + 1])
            else:
                nc.vector.tensor_scalar(out=y2, in0=ps,
                                        scalar1=g.dvsq[:, it:it + 1],
                                        scalar2=None, op0=ALU.mult)
            g.ys2.append(y2)

    def phase_f(g: _GraphState):
        # M2 + b2/d_i accumulated in PSUM; H2 = d*psum in ONE Act op; row
        # norms batched per graph (one sqrt/max/recip); one Y DMA per graph.
        g.oall = opool.tile([128, NT, D_OUT], F32, tag="oall")
        g.ssqall = bvec.tile([128, NT], F32, tag="ssqall")
        for it in range(NT):
            ps = psB.tile([128, D_OUT], F32, tag="psB")
            nc.tensor.matmul(ps, lhsT=g.invd[:, it * 128:(it + 1) * 128],
                             rhs=b2row, start=True, stop=False)
            for jt in range(NT):
                nc.tensor.matmul(ps, lhsT=g.at[:, jt, it * 128:(it + 1) * 128],
                                 rhs=g.ys2[jt], start=False,
                                 stop=(jt == NT - 1))
            h2 = h2pool.tile([128, D_OUT], F32, tag="h2")
            nc.scalar.activation(out=h2, in_=ps, func=AF.Copy,
                                 scale=g.dv[:, it:it + 1])
            g.h2s.append(h2)
            sj2 = sqj.tile([128, D_OUT], F32, tag="sqj2")
            nc.vector.scalar_tensor_tensor(out=sj2, in0=h2, scalar=1.0,
                                           in1=h2, op0=ALU.bypass,
                                           op1=ALU.mult,
                                           accum_out=g.ssqall[:, it:it + 1])
        nrmall = bvec.tile([128, NT], F32, tag="nrmall")
        nc.scalar.sqrt(out=nrmall, in_=g.ssqall)
        clall = bvec.tile([128, NT], F32, tag="clall")
        nc.vector.tensor_scalar_max(clall, nrmall, NORM_EPS)
        invall = bvec.tile([128, NT], F32, tag="invall")
        nc.vector.reciprocal(out=invall, in_=clall)
        for it in range(NT):
            nc.vector.tensor_scalar(out=g.oall[:, it], in0=g.h2s[it],
                                    scalar1=invall[:, it:it + 1],
                                    scalar2=None, op0=ALU.mult)
        nc.sync.dma_start(out=g.Yb.rearrange("(it p) do -> p it do", p=128),
                          in_=g.oall)

    # ---- wave-pipelined driver: all graphs in flight, phase-major -----------
    gs = []
    for bi in range(n_batches):
        g = _GraphState()
        g.XTb, g.XT8b, g.Yb = XT[bi], XT8[bi], Y[bi]
        gs.append(g)

    for g in gs:
        phase_a(g)
    for g in gs:
        phase_b1(g)
    # offset interleave: graph i's gram/threshold overlaps graph i-1's
    # first linear layer on the PE while DVE digests the thresholds.
    for i in range(n_batches):
        phase_b2(gs[i])
        if i >= 1:
            phase_c(gs[i - 1])
    phase_c(gs[n_batches - 1])
    for g in gs:
        phase_d(g)
    for g in gs:
        phase_e(g)
        phase_f(g)


_NC_CACHE = {}


def _get_nc(n_batches: int = BPC):
    if n_batches not in _NC_CACHE:
        _NC_CACHE[n_batches] = build(n_batches)
    return _NC_CACHE[n_batches]


def make_in_maps(X, W1, b1, W2, b2, bpc: int = BPC):
    X = np.asarray(X, dtype=np.float32)
    nb = len(X)
    Xt = X.astype(ml_dtypes.bfloat16).transpose(0, 2, 1)   # [B, D, N] bf16
    XTb16 = np.ascontiguousarray(Xt)
    # DoubleRow pair-interleaved fp8: [b, k, p, i, n], d = k*256 + i*128 + p
    XT8 = np.ascontiguousarray(
        Xt.reshape(nb, KDR, 2, 128, N).transpose(0, 1, 3, 2, 4)
        .astype(ml_dtypes.float8_e4m3))
    W1T = np.ascontiguousarray(
        np.asarray(W1, dtype=np.float32).T.astype(ml_dtypes.bfloat16))
    W2T = np.ascontiguousarray(
        np.asarray(W2, dtype=np.float32).T.astype(ml_dtypes.bfloat16))
    b1 = np.ascontiguousarray(np.asarray(b1, dtype=np.float32))
    b2 = np.ascontiguousarray(np.asarray(b2, dtype=np.float32))
    return [
        {"XT": XTb16[c * bpc:(c + 1) * bpc], "XT8": XT8[c * bpc:(c + 1) * bpc],
         "W1T": W1T, "b1": b1, "W2T": W2T, "b2": b2}
        for c in range(nb // bpc)
    ]


def kernel(X, W1, b1, W2, b2):
    nc = _get_nc()
    in_maps = make_in_maps(X, W1, b1, W2, b2)
    res = run_bass_kernel_spmd(nc, in_maps, core_ids=list(range(N_CORES)))
    return np.concatenate([r["Y"] for r in res.results], axis=0)


# revision 31
# speedup vs baseline: 1.2192x; 1.0030x over previous
"""BatchedGCN Trainium2 kernel.

Per graph (batch element):
  norms_i = ||X_i||;  A = (X@X.T > 0.3*n_i*n_j) + I ; deg = rowsum(A); d = deg^-1/2
  H1 = relu(diag(d) A diag(d) (X @ W1.T) + b1)
  H2 = diag(d) A diag(d) (H1 @ W2.T) + b2
  out = H2 / max(||H2_row||, 1e-12)

Key implementation choices:
- The cosine threshold runs in un-normalized form:
  Xn_i . Xn_j > t  <=>  (X_i . X_j) * (1/max(n_i,eps)) / t > n_j.
  The diag(norm) factor relating X to Xn cancels against the un-normalized
  X used in the first linear layer, so the output path needs no norms.
- The gram matrix G = X X^T runs in fp8 (DoubleRow, 2x rate); the
  thresholding margin is ~40% of the bound while fp8 dot-product error is
  <0.5%, so A is bit-exact.  Row norms are read off G's diagonal blocks
  (computed in a cheap per-row-tile pre-pass), so they are fp8-accurate -
  again only used for the threshold bound.
- The two propagations and both linear layers run in bf16 with fp32 PSUM.
- Bias folding: the layer biases enter the propagation PSUMs as rank-1
  K=1 matmuls b/d_i (using a bounced 1/d row), so each propagation tile
  needs exactly ONE eviction op.  relu(d*u) = d*relu(u) moves the outer
  degree scale of layer 1 into layer 2's eviction scalar (d^2).
- Engine balance: PSUM evictions are only legal on DVE/Act; A-matrix
  self-loop adds run on Pool; all input DMA descriptor generation is on
  the sync HWDGE queue (no compute waits), bounces on gpsimd.
- Phase waves with a b2/c offset interleave keep each graph's latency
  chains (norm -> DRAM-bounce broadcast -> threshold) hidden behind the
  other graphs' dense matmuls, so the PE stays continuously busy.
"""

from contextlib import ExitStack

import ml_dtypes
import numpy as np

import concourse.bass as bass
import concourse.mybir as mybir
import concourse.tile as tile
from concourse import bacc
from concourse.bass_utils import run_bass_kernel_spmd
from concourse.masks import make_identity

B, N, D_IN, D_H, D_OUT = 32, 1024, 768, 256, 128
N_CORES = 8
BPC = B // N_CORES          # graphs per core
NT = N // 128               # 8 row tiles
DTI = D_IN // 128           # 6 input-dim tiles
HC = D_H // 128             # 2 hidden chunks
KDR = D_IN // 256           # 3 DoubleRow K-chunks
F32 = mybir.dt.float32
BF16 = mybir.dt.bfloat16
FP8 = mybir.dt.float8e4

KNN_THRESHOLD = 0.3
COS_EPS = 1e-8
NORM_EPS = 1e-12
ALU = mybir.AluOpType
AF = mybir.ActivationFunctionType
DR = mybir.MatmulPerfMode.DoubleRow




def build(n_batches: int = BPC):
    nc = bacc.Bacc("TRN2", debug=False, num_devices=N_CORES)
    XT = nc.dram_tensor("XT", [n_batches, D_IN, N], BF16, kind="ExternalInput")
    # X^T in fp8, pair-interleaved for DoubleRow: [b, k, p, i, n] with
    # d = k*256 + i*128 + p
    XT8 = nc.dram_tensor("XT8", [n_batches, KDR, 128, 2, N], FP8,
                         kind="ExternalInput")
    W1T = nc.dram_tensor("W1T", [D_IN, D_H], BF16, kind="ExternalInput")
    b1 = nc.dram_tensor("b1", [D_H], F32, kind="ExternalInput")
    W2T = nc.dram_tensor("W2T", [D_H, D_OUT], BF16, kind="ExternalInput")
    b2 = nc.dram_tensor("b2", [D_OUT], F32, kind="ExternalInput")
    Y = nc.dram_tensor("Y", [n_batches, N, D_OUT], F32, kind="ExternalOutput")
    with tile.TileContext(nc) as tc, ExitStack() as ctx:
        _body(ctx, tc, XT.ap(), XT8.ap(), W1T.ap(), b1.ap(), W2T.ap(), b2.ap(),
              Y.ap(), n_batches)
    nc.compile()
    return nc


def _bcast_p(ap: bass.AP, parts: int = 128) -> bass.AP:
    """Broadcast a DRAM AP across `parts` partitions (partition-stride 0)."""
    return bass.AP(tensor=ap.tensor, offset=ap.offset, ap=[[0, parts]] + list(ap.ap))


def _row1(ap: bass.AP, n: int) -> bass.AP:
    """View a flat DRAM AP as a [1, n] row."""
    return bass.AP(tensor=ap.tensor, offset=ap.offset, ap=[[0, 1], [1, n]])


class _GraphState:
    """Per-graph SBUF tiles threaded between pipeline phases."""
    __slots__ = ("XTb", "XT8b", "Yb", "xt", "xt8", "at", "ys1", "ys2",
                 "h1t", "ssqv", "rc03", "ncol16", "sqd16", "nrow", "nrep",
                 "degv", "dv", "dvsq", "invd", "oall", "h2s", "ssqall")


def _body(ctx, tc, XT, XT8, W1T, b1, W2T, b2, Y, n_batches):
    nc = tc.nc

    nb = n_batches
    singles = ctx.enter_context(tc.tile_pool(name="singles", bufs=1))
    sqj = ctx.enter_context(tc.tile_pool(name="sqj", bufs=4))
    xt8pool = ctx.enter_context(tc.tile_pool(name="xt8pool", bufs=nb))
    xtpool = ctx.enter_context(tc.tile_pool(name="xtpool", bufs=2))
    apool = ctx.enter_context(tc.tile_pool(name="apool", bufs=nb))
    bvec = ctx.enter_context(tc.tile_pool(name="bvec", bufs=2 * nb))
    y1pool = ctx.enter_context(tc.tile_pool(name="y1pool", bufs=nb * NT))
    h1pool = ctx.enter_context(tc.tile_pool(name="h1pool", bufs=nb * HC))
    y2pool = ctx.enter_context(tc.tile_pool(name="y2pool", bufs=2 * NT))
    rppool = ctx.enter_context(tc.tile_pool(name="rppool", bufs=4))
    h2pool = ctx.enter_context(tc.tile_pool(name="h2pool", bufs=NT + 4))
    opool = ctx.enter_context(tc.tile_pool(name="opool", bufs=1))
    psA = ctx.enter_context(tc.tile_pool(name="psA", bufs=2, space="PSUM"))
    psB = ctx.enter_context(tc.tile_pool(name="psB", bufs=4, space="PSUM"))

    # ---- one-time constants (plain loads, no prep chains) -------------------
    ident = singles.tile([128, 128], BF16)
    make_identity(nc, ident)
    identf = singles.tile([128, 128], F32)
    make_identity(nc, identf)

    # bias rows in partition 0 (bf16, gpsimd DMA casts f32 -> bf16)
    b1row = singles.tile([1, D_H], BF16, tag="b1row")
    nc.gpsimd.dma_start(out=b1row, in_=_row1(b1, D_H))
    b2row = singles.tile([1, D_OUT], BF16, tag="b2row")
    nc.gpsimd.dma_start(out=b2row, in_=_row1(b2, D_OUT))
    ones1 = singles.tile([1, 128], BF16, tag="ones1")
    nc.gpsimd.memset(ones1, 1.0)

    w1t = singles.tile([128, DTI, D_H], BF16, tag="w1t")
    nc.sync.dma_start(out=w1t, in_=W1T.rearrange("(dt p) h -> p dt h", p=128))
    w2t = singles.tile([128, HC, D_OUT], BF16, tag="w2t")
    nc.sync.dma_start(out=w2t, in_=W2T.rearrange("(k p) do -> p k do", p=128))

    inv_t = 1.0 / KNN_THRESHOLD

    # ---- per-phase emitters -------------------------------------------------
    def phase_a(g: _GraphState):
        # All input loads issued upfront on the sync HWDGE queue, which
        # carries no compute-dependent waits (bounces live on gpsimd), so
        # transfers stream from t=0.
        g.xt8 = xt8pool.tile([128, KDR, 2, N], FP8, tag="xt8")
        nc.sync.dma_start(out=g.xt8, in_=g.XT8b.rearrange("k p i n -> p k i n"))
        g.xt = xtpool.tile([128, DTI, N], BF16, tag="xt")
        nc.sync.dma_start(out=g.xt, in_=g.XTb.rearrange("(dt p) n -> p dt n",
                                                        p=128))
        g.at = None
        g.ys1 = []
        g.ys2 = []
        g.h1t = []
        g.h2s = []

    def phase_b1(g: _GraphState):
        # pre-pass: row norms from the gram diagonal blocks; then the
        # norm -> reciprocal chain and the DRAM-bounce broadcast of n_j.
        # Emitted for all graphs before any gram so the bounce round-trip
        # hides behind the other graphs' pre-passes on the PE.
        g.ssqv = bvec.tile([128, NT], F32, tag="ssqv")
        for it in range(NT):
            psd = psB.tile([128, D_OUT], F32, tag="psB", name="psd")
            blk = slice(it * 128, (it + 1) * 128)
            for k in range(KDR):
                nc.tensor.matmul(psd, lhsT=g.xt8[:, k, :, blk],
                                 rhs=g.xt8[:, k, :, blk],
                                 start=(k == 0), stop=(k == KDR - 1),
                                 perf_mode=DR)
            dj = sqj.tile([128, 128], BF16, tag="dj")
            nc.vector.scalar_tensor_tensor(
                out=dj, in0=psd, scalar=1.0, in1=identf,
                op0=ALU.bypass, op1=ALU.mult,
                accum_out=g.ssqv[:, it:it + 1])
        ncol = bvec.tile([128, NT], F32, tag="ncol")
        nc.scalar.sqrt(out=ncol, in_=g.ssqv)
        nclamp = bvec.tile([128, NT], F32, tag="nclamp")
        nc.vector.tensor_scalar_max(nclamp, ncol, COS_EPS)
        rcol = bvec.tile([128, NT], F32, tag="rcol")
        nc.vector.reciprocal(out=rcol, in_=nclamp)
        g.rc03 = bvec.tile([128, NT], F32, tag="rc03")
        nc.vector.tensor_scalar_mul(g.rc03, rcol, inv_t)
        g.ncol16 = bvec.tile([128, NT], BF16, tag="ncol16")
        nc.scalar.activation(out=g.ncol16, in_=ncol, func=AF.Copy)

    def phase_b2(g: _GraphState):
        # on-chip row broadcast of n_j: per-column PE transposes gather the
        # norm column into a partition-0 PSUM row; K=1 rank-1 matmuls with a
        # ones row replicate it across all partitions.  No DRAM bounce.
        psrowN = psA.tile([1, NT, 128], BF16, tag="psA", name="psrowN")
        for it in range(NT):
            nc.tensor.transpose(out=psrowN[:, it], in_=g.ncol16[:, it:it + 1],
                                identity=ident)
        g.nrow = bvec.tile([1, NT, 128], BF16, tag="nrow")
        nc.scalar.activation(out=g.nrow, in_=psrowN, func=AF.Copy)
        psN = psA.tile([128, N], F32, tag="psA", name="psN")
        for it in range(NT):
            nc.tensor.matmul(psN[:, it * 128:(it + 1) * 128], lhsT=ones1,
                             rhs=g.nrow[:, it], start=True, stop=True)
        g.nrep = rppool.tile([128, N], BF16, tag="nrep")
        nc.scalar.activation(out=g.nrep, in_=psN, func=AF.Copy)

        # main pass: paired two-bank G row PSUMs -> one threshold eviction
        # per row tile -> A (+ self loop on Pool), deg row-sums fused.
        g.at = apool.tile([128, NT, N], BF16, tag="at")
        g.degv = bvec.tile([128, NT], F32, tag="degv")
        for it in range(NT):
            a_t = g.at[:, it]
            ps = psA.tile([128, N], F32, tag="psA")
            for jh in range(2):
                psh = ps[:, jh * 512:(jh + 1) * 512]
                for k in range(KDR):
                    nc.tensor.matmul(
                        psh, lhsT=g.xt8[:, k, :, it * 128:(it + 1) * 128],
                        rhs=g.xt8[:, k, :, jh * 512:(jh + 1) * 512],
                        start=(k == 0), stop=(k == KDR - 1), perf_mode=DR)
            nc.vector.scalar_tensor_tensor(
                out=a_t, in0=ps, scalar=g.rc03[:, it:it + 1], in1=g.nrep,
                op0=ALU.mult, op1=ALU.is_gt,
                accum_out=g.degv[:, it:it + 1])
            nc.gpsimd.tensor_tensor(out=a_t[:, it * 128:(it + 1) * 128],
                                    in0=a_t[:, it * 128:(it + 1) * 128],
                                    in1=ident, op=ALU.add)

        # deg -> d = deg^-1/2 (and 1/d as bf16 column, d^2)
        sqd = bvec.tile([128, NT], F32, tag="sqd")
        nc.scalar.activation(out=sqd, in_=g.degv, func=AF.Sqrt, bias=1.0)
        g.dv = bvec.tile([128, NT], F32, tag="dv")
        nc.vector.reciprocal(out=g.dv, in_=sqd)
        g.dvsq = bvec.tile([128, NT], F32, tag="dvsq")
        nc.vector.tensor_tensor(out=g.dvsq, in0=g.dv, in1=g.dv, op=ALU.mult)
        g.sqd16 = bvec.tile([128, NT], BF16, tag="sqd16")
        nc.scalar.activation(out=g.sqd16, in_=sqd, func=AF.Copy)

    def phase_c(g: _GraphState):
        # 1/d as a [1, N] partition-0 row (for the rank-1 bias folds of
        # both layers), via the same on-chip transpose trick.  Emitted here
        # (one wave after b2) so the deg chain is long complete.
        psrowI = psA.tile([1, NT, 128], BF16, tag="psA", name="psrowI")
        for it in range(NT):
            nc.tensor.transpose(out=psrowI[:, it], in_=g.sqd16[:, it:it + 1],
                                identity=ident)
        g.invd = rppool.tile([1, NT, 128], BF16, tag="invd")
        nc.scalar.activation(out=g.invd, in_=psrowI, func=AF.Copy)

        # G1 = X @ W1.T [n, h]; evict scaled by d -> Ys1 bf16 (Act engine).
        for it in range(NT):
            ps = psB.tile([128, D_H], F32, tag="psB")
            for dt in range(DTI):
                nc.tensor.matmul(ps, lhsT=g.xt[:, dt, it * 128:(it + 1) * 128],
                                 rhs=w1t[:, dt], start=(dt == 0),
                                 stop=(dt == DTI - 1))
            y1 = y1pool.tile([128, D_H], BF16, tag="y1")
            nc.scalar.activation(out=y1, in_=ps, func=AF.Copy,
                                 scale=g.dv[:, it:it + 1])
            g.ys1.append(y1)

    def phase_d(g: _GraphState):
        # M1^T + b1/d_i = (A diag(d) G1)^T + outer(b1, 1/d) over paired
        # two-bank PSUMs (one per hc); H1'^T = relu(psum) in ONE Act op.
        # The missing outer diag(d) resurfaces as d^2 in phase_e's scalar.
        for hc in range(HC):
            g.h1t.append(h1pool.tile([128, N], BF16, tag="h1", name="h1"))
        for hc in range(HC):
            pss = psA.tile([128, N], F32, tag="psA", name="psd2")
            for ih in range(2):
                nc.tensor.matmul(pss[:, ih * 512:(ih + 1) * 512],
                                 lhsT=b1row[:, hc * 128:(hc + 1) * 128],
                                 rhs=g.invd[:, 4 * ih:4 * (ih + 1)],
                                 start=True, stop=False)
            for jt in range(NT):
                sp = jt == NT - 1
                lhsT = g.ys1[jt][:, hc * 128:(hc + 1) * 128]
                for ih in range(2):
                    nc.tensor.matmul(pss[:, ih * 512:(ih + 1) * 512],
                                     lhsT=lhsT,
                                     rhs=g.at[:, jt, ih * 512:(ih + 1) * 512],
                                     start=False, stop=sp)
            nc.scalar.activation(out=g.h1t[hc], in_=pss, func=AF.Relu)

    def phase_e(g: _GraphState):
        # Ys2 = d^2 * (H1' @ W2.T)  (the d^2 restores both diag(d) factors)
        for it in range(NT):
            ps = psB.tile([128, D_OUT], F32, tag="psB")
            for hc in range(HC):
                nc.tensor.matmul(ps, lhsT=g.h1t[hc][:, it * 128:(it + 1) * 128],
                                 rhs=w2t[:, hc], start=(hc == 0),
                                 stop=(hc == HC - 1))
            y2 = y2pool.tile([128, D_OUT], BF16, tag="y2")
            if it % 2 == 0:
                nc.scalar.activation(out=y2, in_=ps, func=AF.Copy,
                                     scale=g.dvsq[:, it:it + 1])
            else:
                nc.vector.tensor_scalar(out=y2, in0=ps,
                                        scalar1=g.dvsq[:, it:it + 1],
                                        scalar2=None, op0=ALU.mult)
            g.ys2.append(y2)

    def phase_f(g: _GraphState):
        # M2 + b2/d_i accumulated in PSUM; H2 = d*psum in ONE Act op; row
        # norms batched per graph (one sqrt/max/recip); one Y DMA per graph.
        g.oall = opool.tile([128, NT, D_OUT], F32, tag="oall")
        g.ssqall = bvec.tile([128, NT], F32, tag="ssqall")
        for it in range(NT):
            ps = psB.tile([128, D_OUT], F32, tag="psB")
            nc.tensor.matmul(ps, lhsT=g.invd[:, it], rhs=b2row,
                             start=True, stop=False)
            for jt in range(NT):
                nc.tensor.matmul(ps, lhsT=g.at[:, jt, it * 128:(it + 1) * 128],
                                 rhs=g.ys2[jt], start=False,
                                 stop=(jt == NT - 1))
            h2 = h2pool.tile([128, D_OUT], F32, tag="h2")
            nc.scalar.activation(out=h2, in_=ps, func=AF.Copy,
                                 scale=g.dv[:, it:it + 1])
            g.h2s.append(h2)
            sj2 = sqj.tile([128, D_OUT], F32, tag="sqj2")
            nc.vector.scalar_tensor_tensor(out=sj2, in0=h2, scalar=1.0,
                                           in1=h2, op0=ALU.bypass,
                                           op1=ALU.mult,
                                           accum_out=g.ssqall[:, it:it + 1])
        nrmall = bvec.tile([128, NT], F32, tag="nrmall")
        nc.scalar.sqrt(out=nrmall, in_=g.ssqall)
        clall = bvec.tile([128, NT], F32, tag="clall")
        nc.vector.tensor_scalar_max(clall, nrmall, NORM_EPS)
        invall = bvec.tile([128, NT], F32, tag="invall")
        nc.vector.reciprocal(out=invall, in_=clall)
        for it in range(NT):
            nc.vector.tensor_scalar(out=g.oall[:, it], in0=g.h2s[it],
                                    scalar1=invall[:, it:it + 1],
                                    scalar2=None, op0=ALU.mult)
        nc.sync.dma_start(out=g.Yb.rearrange("(it p) do -> p it do", p=128),
                          in_=g.oall)

    # ---- wave-pipelined driver: all graphs in flight, phase-major -----------
    gs = []
    for bi in range(n_batches):
        g = _GraphState()
        g.XTb, g.XT8b, g.Yb = XT[bi], XT8[bi], Y[bi]
        gs.append(g)

    for g in gs:
        phase_a(g)
    for g in gs:
        phase_b1(g)
    # offset interleave: graph i's gram/threshold overlaps graph i-1's
    # first linear layer on the PE while DVE digests the thresholds.
    for i in range(n_batches):
        phase_b2(gs[i])
        if i >= 1:
            phase_c(gs[i - 1])
    phase_c(gs[n_batches - 1])
    for g in gs:
        phase_d(g)
    for g in gs:
        phase_e(g)
        phase_f(g)


_NC_CACHE = {}


def _get_nc(n_batches: int = BPC):
    if n_batches not in _NC_CACHE:
        _NC_CACHE[n_batches] = build(n_batches)
    return _NC_CACHE[n_batches]


def make_in_maps(X, W1, b1, W2, b2, bpc: int = BPC):
    X = np.asarray(X, dtype=np.float32)
    nb = len(X)
    Xt = X.astype(ml_dtypes.bfloat16).transpose(0, 2, 1)   # [B, D, N] bf16
    XTb16 = np.ascontiguousarray(Xt)
    # DoubleRow pair-interleaved fp8: [b, k, p, i, n], d = k*256 + i*128 + p
    XT8 = np.ascontiguousarray(
        Xt.reshape(nb, KDR, 2, 128, N).transpose(0, 1, 3, 2, 4)
        .astype(ml_dtypes.float8_e4m3))
    W1T = np.ascontiguousarray(
        np.asarray(W1, dtype=np.float32).T.astype(ml_dtypes.bfloat16))
    W2T = np.ascontiguousarray(
        np.asarray(W2, dtype=np.float32).T.astype(ml_dtypes.bfloat16))
    b1 = np.ascontiguousarray(np.asarray(b1, dtype=np.float32))
    b2 = np.ascontiguousarray(np.asarray(b2, dtype=np.float32))
    return [
        {"XT": XTb16[c * bpc:(c + 1) * bpc], "XT8": XT8[c * bpc:(c + 1) * bpc],
         "W1T": W1T, "b1": b1, "W2T": W2T, "b2": b2}
        for c in range(nb // bpc)
    ]


def kernel(X, W1, b1, W2, b2):
    nc = _get_nc()
    in_maps = make_in_maps(X, W1, b1, W2, b2)
    res = run_bass_kernel_spmd(nc, in_maps, core_ids=list(range(N_CORES)))
    return np.concatenate([r["Y"] for r in res.results], axis=0)


# revision 33
# speedup vs baseline: 1.2520x; 1.0269x over previous
"""BatchedGCN Trainium2 kernel.

Per graph (batch element):
  norms_i = ||X_i||;  A = (X@X.T > 0.3*n_i*n_j) + I ; deg = rowsum(A); d = deg^-1/2
  H1 = relu(diag(d) A diag(d) (X @ W1.T) + b1)
  H2 = diag(d) A diag(d) (H1 @ W2.T) + b2
  out = H2 / max(||H2_row||, 1e-12)

Key implementation choices:
- The cosine threshold runs in un-normalized form:
  Xn_i . Xn_j > t  <=>  (X_i . X_j) * (1/max(n_i,eps)) / t > n_j.
  The diag(norm) factor relating X to Xn cancels against the un-normalized
  X used in the first linear layer, so the output path needs no norms.
- The gram matrix G = X X^T runs in fp8 (DoubleRow, 2x rate); the
  thresholding margin is ~40% of the bound while fp8 dot-product error is
  <0.5%, so A is bit-exact.  Row norms are read off G's diagonal blocks
  (computed in a cheap per-row-tile pre-pass), so they are fp8-accurate -
  again only used for the threshold bound.
- The two propagations and both linear layers run in bf16 with fp32 PSUM.
- Bias folding: the layer biases enter the propagation PSUMs as rank-1
  K=1 matmuls b/d_i (using a bounced 1/d row), so each propagation tile
  needs exactly ONE eviction op.  relu(d*u) = d*relu(u) moves the outer
  degree scale of layer 1 into layer 2's eviction scalar (d^2).
- Engine balance: PSUM evictions are only legal on DVE/Act; A-matrix
  self-loop adds run on Pool; all input DMA descriptor generation is on
  the sync HWDGE queue (no compute waits), bounces on gpsimd.
- Phase waves with a b2/c offset interleave keep each graph's latency
  chains (norm -> DRAM-bounce broadcast -> threshold) hidden behind the
  other graphs' dense matmuls, so the PE stays continuously busy.
"""

from contextlib import ExitStack

import ml_dtypes
import numpy as np

import concourse.bass as bass
import concourse.mybir as mybir
import concourse.tile as tile
from concourse import bacc
from concourse.bass_utils import run_bass_kernel_spmd
from concourse.masks import make_identity

B, N, D_IN, D_H, D_OUT = 32, 1024, 768, 256, 128
N_CORES = 8
BPC = B // N_CORES          # graphs per core
NT = N // 128               # 8 row tiles
DTI = D_IN // 128           # 6 input-dim tiles
HC = D_H // 128             # 2 hidden chunks
KDR = D_IN // 256           # 3 DoubleRow K-chunks
F32 = mybir.dt.float32
BF16 = mybir.dt.bfloat16
FP8 = mybir.dt.float8e4

KNN_THRESHOLD = 0.3
COS_EPS = 1e-8
NORM_EPS = 1e-12
ALU = mybir.AluOpType
AF = mybir.ActivationFunctionType
DR = mybir.MatmulPerfMode.DoubleRow




def build(n_batches: int = BPC):
    nc = bacc.Bacc("TRN2", debug=False, num_devices=N_CORES)
    XT = nc.dram_tensor("XT", [n_batches, D_IN, N], BF16, kind="ExternalInput")
    # X^T in fp8, pair-interleaved for DoubleRow: [b, k, p, i, n] with
    # d = k*256 + i*128 + p
    XT8 = nc.dram_tensor("XT8", [n_batches, KDR, 128, 2, N], FP8,
                         kind="ExternalInput")
    W1T = nc.dram_tensor("W1T", [D_IN, D_H], BF16, kind="ExternalInput")
    b1 = nc.dram_tensor("b1", [D_H], F32, kind="ExternalInput")
    W2T = nc.dram_tensor("W2T", [D_H, D_OUT], BF16, kind="ExternalInput")
    b2 = nc.dram_tensor("b2", [D_OUT], F32, kind="ExternalInput")
    Y = nc.dram_tensor("Y", [n_batches, N, D_OUT], F32, kind="ExternalOutput")
    with tile.TileContext(nc) as tc, ExitStack() as ctx:
        _body(ctx, tc, XT.ap(), XT8.ap(), W1T.ap(), b1.ap(), W2T.ap(), b2.ap(),
              Y.ap(), n_batches)
    nc.compile()
    return nc


def _bcast_p(ap: bass.AP, parts: int = 128) -> bass.AP:
    """Broadcast a DRAM AP across `parts` partitions (partition-stride 0)."""
    return bass.AP(tensor=ap.tensor, offset=ap.offset, ap=[[0, parts]] + list(ap.ap))


def _row1(ap: bass.AP, n: int) -> bass.AP:
    """View a flat DRAM AP as a [1, n] row."""
    return bass.AP(tensor=ap.tensor, offset=ap.offset, ap=[[0, 1], [1, n]])


class _GraphState:
    """Per-graph SBUF tiles threaded between pipeline phases."""
    __slots__ = ("XTb", "XT8b", "Yb", "xt", "xt8", "at", "ys1", "ys2",
                 "h1t", "ssqv", "rc03", "ncol16", "sqd16", "nrow", "nrep",
                 "degv", "dv", "dvsq", "invd", "oall", "h2s", "ssqall")


def _body(ctx, tc, XT, XT8, W1T, b1, W2T, b2, Y, n_batches):
    nc = tc.nc

    nb = n_batches
    singles = ctx.enter_context(tc.tile_pool(name="singles", bufs=1))
    sqj = ctx.enter_context(tc.tile_pool(name="sqj", bufs=4))
    xt8pool = ctx.enter_context(tc.tile_pool(name="xt8pool", bufs=nb))
    xtpool = ctx.enter_context(tc.tile_pool(name="xtpool", bufs=2))
    apool = ctx.enter_context(tc.tile_pool(name="apool", bufs=nb))
    bvec = ctx.enter_context(tc.tile_pool(name="bvec", bufs=2 * nb))
    y1pool = ctx.enter_context(tc.tile_pool(name="y1pool", bufs=nb * NT))
    h1pool = ctx.enter_context(tc.tile_pool(name="h1pool", bufs=nb * HC))
    y2pool = ctx.enter_context(tc.tile_pool(name="y2pool", bufs=2 * NT))
    rppool = ctx.enter_context(tc.tile_pool(name="rppool", bufs=4))
    h2pool = ctx.enter_context(tc.tile_pool(name="h2pool", bufs=NT + 4))
    opool = ctx.enter_context(tc.tile_pool(name="opool", bufs=1))
    psA = ctx.enter_context(tc.tile_pool(name="psA", bufs=2, space="PSUM"))
    psB = ctx.enter_context(tc.tile_pool(name="psB", bufs=4, space="PSUM"))

    # ---- one-time constants (plain loads, no prep chains) -------------------
    ident = singles.tile([128, 128], BF16)
    make_identity(nc, ident)
    identf = singles.tile([128, 128], F32)
    make_identity(nc, identf)

    # bias rows in partition 0 (bf16, gpsimd DMA casts f32 -> bf16)
    b1row = singles.tile([1, D_H], BF16, tag="b1row")
    nc.gpsimd.dma_start(out=b1row, in_=_row1(b1, D_H))
    b2row = singles.tile([1, D_OUT], BF16, tag="b2row")
    nc.gpsimd.dma_start(out=b2row, in_=_row1(b2, D_OUT))
    ones1 = singles.tile([1, 128], BF16, tag="ones1")
    nc.gpsimd.memset(ones1, 1.0)

    w1t = singles.tile([128, DTI, D_H], BF16, tag="w1t")
    nc.sync.dma_start(out=w1t, in_=W1T.rearrange("(dt p) h -> p dt h", p=128))
    w2t = singles.tile([128, HC, D_OUT], BF16, tag="w2t")
    nc.sync.dma_start(out=w2t, in_=W2T.rearrange("(k p) do -> p k do", p=128))

    inv_t = 1.0 / KNN_THRESHOLD

    # ---- per-phase emitters -------------------------------------------------
    def phase_a(g: _GraphState):
        # All input loads issued upfront on the sync HWDGE queue, which
        # carries no compute-dependent waits (bounces live on gpsimd), so
        # transfers stream from t=0.
        g.xt8 = xt8pool.tile([128, KDR, 2, N], FP8, tag="xt8")
        nc.sync.dma_start(out=g.xt8, in_=g.XT8b.rearrange("k p i n -> p k i n"))
        g.xt = xtpool.tile([128, DTI, N], BF16, tag="xt")
        nc.sync.dma_start(out=g.xt, in_=g.XTb.rearrange("(dt p) n -> p dt n",
                                                        p=128))
        g.at = None
        g.ys1 = []
        g.ys2 = []
        g.h1t = []
        g.h2s = []

    def phase_b1(g: _GraphState):
        # pre-pass: row norms from the gram diagonal blocks; then the
        # norm -> reciprocal chain and the DRAM-bounce broadcast of n_j.
        # Emitted for all graphs before any gram so the bounce round-trip
        # hides behind the other graphs' pre-passes on the PE.
        g.ssqv = bvec.tile([128, NT], F32, tag="ssqv")
        for it in range(NT):
            psd = psB.tile([128, D_OUT], F32, tag="psB", name="psd")
            blk = slice(it * 128, (it + 1) * 128)
            for k in range(KDR):
                nc.tensor.matmul(psd, lhsT=g.xt8[:, k, :, blk],
                                 rhs=g.xt8[:, k, :, blk],
                                 start=(k == 0), stop=(k == KDR - 1),
                                 perf_mode=DR)
            dj = sqj.tile([128, 128], BF16, tag="dj")
            nc.vector.scalar_tensor_tensor(
                out=dj, in0=psd, scalar=1.0, in1=identf,
                op0=ALU.bypass, op1=ALU.mult,
                accum_out=g.ssqv[:, it:it + 1])
        ncol = bvec.tile([128, NT], F32, tag="ncol")
        nc.scalar.sqrt(out=ncol, in_=g.ssqv)
        nclamp = bvec.tile([128, NT], F32, tag="nclamp")
        nc.vector.tensor_scalar_max(nclamp, ncol, COS_EPS)
        rcol = bvec.tile([128, NT], F32, tag="rcol")
        nc.vector.reciprocal(out=rcol, in_=nclamp)
        g.rc03 = bvec.tile([128, NT], F32, tag="rc03")
        nc.vector.tensor_scalar_mul(g.rc03, rcol, inv_t)
        g.ncol16 = bvec.tile([128, NT], BF16, tag="ncol16")
        nc.scalar.activation(out=g.ncol16, in_=ncol, func=AF.Copy)

    def phase_c_invd(g: _GraphState):
        # 1/d as a [1, N] partition-0 row (for the rank-1 bias folds of
        # both layers), via the on-chip transpose trick.  Emitted one wave
        # after b2 so the deg chain is long complete.
        psrowI = psA.tile([1, NT, 128], BF16, tag="psA", name="psrowI")
        for it in range(NT):
            nc.tensor.transpose(out=psrowI[:, it], in_=g.sqd16[:, it:it + 1],
                                identity=ident)
        g.invd = rppool.tile([1, NT, 128], BF16, tag="invd")
        nc.scalar.activation(out=g.invd, in_=psrowI, func=AF.Copy)

    def phase_c_group(g: _GraphState, it: int):
        # one row tile of G1 = X @ W1.T; evict scaled by d -> Ys1 bf16 (Act)
        ps = psB.tile([128, D_H], F32, tag="psB")
        for dt in range(DTI):
            nc.tensor.matmul(ps, lhsT=g.xt[:, dt, it * 128:(it + 1) * 128],
                             rhs=w1t[:, dt], start=(dt == 0),
                             stop=(dt == DTI - 1))
        y1 = y1pool.tile([128, D_H], BF16, tag="y1")
        nc.scalar.activation(out=y1, in_=ps, func=AF.Copy,
                             scale=g.dv[:, it:it + 1])
        g.ys1.append(y1)

    def phase_b2(g: _GraphState, gp: _GraphState | None):
        # on-chip row broadcast of n_j: per-column PE transposes gather the
        # norm column into a partition-0 PSUM row; K=1 rank-1 matmuls with a
        # ones row replicate it across all partitions.  No DRAM bounce.
        psrowN = psA.tile([1, NT, 128], BF16, tag="psA", name="psrowN")
        for it in range(NT):
            nc.tensor.transpose(out=psrowN[:, it], in_=g.ncol16[:, it:it + 1],
                                identity=ident)
        g.nrow = bvec.tile([1, NT, 128], BF16, tag="nrow")
        nc.scalar.activation(out=g.nrow, in_=psrowN, func=AF.Copy)
        psN = psA.tile([128, N], F32, tag="psA", name="psN")
        for it in range(NT):
            nc.tensor.matmul(psN[:, it * 128:(it + 1) * 128], lhsT=ones1,
                             rhs=g.nrow[:, it], start=True, stop=True)
        g.nrep = rppool.tile([128, N], BF16, tag="nrep")
        nc.scalar.activation(out=g.nrep, in_=psN, func=AF.Copy)
        if gp is not None:
            phase_c_invd(gp)

        # main pass: paired two-bank G row PSUMs -> one threshold eviction
        # per row tile -> A (+ self loop on Pool), deg row-sums fused.
        # Each gram tile is paired with one of the previous graph's first
        # linear layer tiles on the PE, so the DVE threshold (the slower
        # drain) fully hides behind PE work.
        g.at = apool.tile([128, NT, N], BF16, tag="at")
        g.degv = bvec.tile([128, NT], F32, tag="degv")
        for it in range(NT):
            a_t = g.at[:, it]
            ps = psA.tile([128, N], F32, tag="psA")
            for jh in range(2):
                psh = ps[:, jh * 512:(jh + 1) * 512]
                for k in range(KDR):
                    nc.tensor.matmul(
                        psh, lhsT=g.xt8[:, k, :, it * 128:(it + 1) * 128],
                        rhs=g.xt8[:, k, :, jh * 512:(jh + 1) * 512],
                        start=(k == 0), stop=(k == KDR - 1), perf_mode=DR)
            nc.vector.scalar_tensor_tensor(
                out=a_t, in0=ps, scalar=g.rc03[:, it:it + 1], in1=g.nrep,
                op0=ALU.mult, op1=ALU.is_gt,
                accum_out=g.degv[:, it:it + 1])
            nc.gpsimd.tensor_tensor(out=a_t[:, it * 128:(it + 1) * 128],
                                    in0=a_t[:, it * 128:(it + 1) * 128],
                                    in1=ident, op=ALU.add)
            if gp is not None:
                phase_c_group(gp, it)

        # deg -> d = deg^-1/2 (and 1/d as bf16 column, d^2)
        sqd = bvec.tile([128, NT], F32, tag="sqd")
        nc.scalar.activation(out=sqd, in_=g.degv, func=AF.Sqrt, bias=1.0)
        g.dv = bvec.tile([128, NT], F32, tag="dv")
        nc.vector.reciprocal(out=g.dv, in_=sqd)
        g.dvsq = bvec.tile([128, NT], F32, tag="dvsq")
        nc.vector.tensor_tensor(out=g.dvsq, in0=g.dv, in1=g.dv, op=ALU.mult)
        g.sqd16 = bvec.tile([128, NT], BF16, tag="sqd16")
        nc.scalar.activation(out=g.sqd16, in_=sqd, func=AF.Copy)

    def phase_d(g: _GraphState):
        # M1^T + b1/d_i = (A diag(d) G1)^T + outer(b1, 1/d) over paired
        # two-bank PSUMs (one per hc); H1'^T = relu(psum) in ONE Act op.
        # The missing outer diag(d) resurfaces as d^2 in phase_e's scalar.
        for hc in range(HC):
            g.h1t.append(h1pool.tile([128, N], BF16, tag="h1", name="h1"))
        for hc in range(HC):
            pss = psA.tile([128, N], F32, tag="psA", name="psd2")
            for ih in range(2):
                nc.tensor.matmul(pss[:, ih * 512:(ih + 1) * 512],
                                 lhsT=b1row[:, hc * 128:(hc + 1) * 128],
                                 rhs=g.invd[:, 4 * ih:4 * (ih + 1)],
                                 start=True, stop=False)
            for jt in range(NT):
                sp = jt == NT - 1
                lhsT = g.ys1[jt][:, hc * 128:(hc + 1) * 128]
                for ih in range(2):
                    nc.tensor.matmul(pss[:, ih * 512:(ih + 1) * 512],
                                     lhsT=lhsT,
                                     rhs=g.at[:, jt, ih * 512:(ih + 1) * 512],
                                     start=False, stop=sp)
            nc.scalar.activation(out=g.h1t[hc], in_=pss, func=AF.Relu)

    def phase_e(g: _GraphState):
        # Ys2 = d^2 * (H1' @ W2.T)  (the d^2 restores both diag(d) factors)
        for it in range(NT):
            ps = psB.tile([128, D_OUT], F32, tag="psB")
            for hc in range(HC):
                nc.tensor.matmul(ps, lhsT=g.h1t[hc][:, it * 128:(it + 1) * 128],
                                 rhs=w2t[:, hc], start=(hc == 0),
                                 stop=(hc == HC - 1))
            y2 = y2pool.tile([128, D_OUT], BF16, tag="y2")
            if it % 2 == 0:
                nc.scalar.activation(out=y2, in_=ps, func=AF.Copy,
                                     scale=g.dvsq[:, it:it + 1])
            else:
                nc.vector.tensor_scalar(out=y2, in0=ps,
                                        scalar1=g.dvsq[:, it:it + 1],
                                        scalar2=None, op0=ALU.mult)
            g.ys2.append(y2)

    def phase_f(g: _GraphState):
        # M2 + b2/d_i accumulated in PSUM; H2 = d*psum in ONE Act op; row
        # norms batched per graph (one sqrt/max/recip); one Y DMA per graph.
        g.oall = opool.tile([128, NT, D_OUT], F32, tag="oall")
        g.ssqall = bvec.tile([128, NT], F32, tag="ssqall")
        for it in range(NT):
            ps = psB.tile([128, D_OUT], F32, tag="psB")
            nc.tensor.matmul(ps, lhsT=g.invd[:, it], rhs=b2row,
                             start=True, stop=False)
            for jt in range(NT):
                nc.tensor.matmul(ps, lhsT=g.at[:, jt, it * 128:(it + 1) * 128],
                                 rhs=g.ys2[jt], start=False,
                                 stop=(jt == NT - 1))
            h2 = h2pool.tile([128, D_OUT], F32, tag="h2")
            nc.scalar.activation(out=h2, in_=ps, func=AF.Copy,
                                 scale=g.dv[:, it:it + 1])
            g.h2s.append(h2)
            sj2 = sqj.tile([128, D_OUT], F32, tag="sqj2")
            nc.vector.scalar_tensor_tensor(out=sj2, in0=h2, scalar=1.0,
                                           in1=h2, op0=ALU.bypass,
                                           op1=ALU.mult,
                                           accum_out=g.ssqall[:, it:it + 1])
        nrmall = bvec.tile([128, NT], F32, tag="nrmall")
        nc.scalar.sqrt(out=nrmall, in_=g.ssqall)
        clall = bvec.tile([128, NT], F32, tag="clall")
        nc.vector.tensor_scalar_max(clall, nrmall, NORM_EPS)
        invall = bvec.tile([128, NT], F32, tag="invall")
        nc.vector.reciprocal(out=invall, in_=clall)
        for it in range(NT):
            nc.vector.tensor_scalar(out=g.oall[:, it], in0=g.h2s[it],
                                    scalar1=invall[:, it:it + 1],
                                    scalar2=None, op0=ALU.mult)
        nc.sync.dma_start(out=g.Yb.rearrange("(it p) do -> p it do", p=128),
                          in_=g.oall)

    # ---- wave-pipelined driver: all graphs in flight, phase-major -----------
    gs = []
    for bi in range(n_batches):
        g = _GraphState()
        g.XTb, g.XT8b, g.Yb = XT[bi], XT8[bi], Y[bi]
        gs.append(g)

    for g in gs:
        phase_a(g)
    for g in gs:
        phase_b1(g)
    # offset interleave: graph i's gram/threshold tiles alternate with
    # graph i-1's first-linear-layer tiles on the PE, so DVE thresholds
    # (the slower drain) hide behind PE work.
    for i in range(n_batches):
        phase_b2(gs[i], gs[i - 1] if i >= 1 else None)
    gl = gs[n_batches - 1]
    phase_c_invd(gl)
    for it in range(NT):
        phase_c_group(gl, it)
    for g in gs:
        phase_d(g)
    for g in gs:
        phase_e(g)
        phase_f(g)


_NC_CACHE = {}


def _get_nc(n_batches: int = BPC):
    if n_batches not in _NC_CACHE:
        _NC_CACHE[n_batches] = build(n_batches)
    return _NC_CACHE[n_batches]


def make_in_maps(X, W1, b1, W2, b2, bpc: int = BPC):
    X = np.asarray(X, dtype=np.float32)
    nb = len(X)
    Xt = X.astype(ml_dtypes.bfloat16).transpose(0, 2, 1)   # [B, D, N] bf16
    XTb16 = np.ascontiguousarray(Xt)
    # DoubleRow pair-interleaved fp8: [b, k, p, i, n], d = k*256 + i*128 + p
    XT8 = np.ascontiguousarray(
        Xt.reshape(nb, KDR, 2, 128, N).transpose(0, 1, 3, 2, 4)
        .astype(ml_dtypes.float8_e4m3))
    W1T = np.ascontiguousarray(
        np.asarray(W1, dtype=np.float32).T.astype(ml_dtypes.bfloat16))
    W2T = np.ascontiguousarray(
        np.asarray(W2, dtype=np.float32).T.astype(ml_dtypes.bfloat16))
    b1 = np.ascontiguousarray(np.asarray(b1, dtype=np.float32))
    b2 = np.ascontiguousarray(np.asarray(b2, dtype=np.float32))
    return [
        {"XT": XTb16[c * bpc:(c + 1) * bpc], "XT8": XT8[c * bpc:(c + 1) * bpc],
         "W1T": W1T, "b1": b1, "W2T": W2T, "b2": b2}
        for c in range(nb // bpc)
    ]


def kernel(X, W1, b1, W2, b2):
    nc = _get_nc()
    in_maps = make_in_maps(X, W1, b1, W2, b2)
    res = run_bass_kernel_spmd(nc, in_maps, core_ids=list(range(N_CORES)))
    return np.concatenate([r["Y"] for r in res.results], axis=0)
